# revision 13
# baseline (speedup 1.0000x reference)
"""Causal multi-head self-attention block (B=2, T=2048, C=1024, H=16) on 8
Trainium2 NeuronCores.

Sharding: core c = 4*b + g handles batch b (2-way data parallel) and head
group g (4-way tensor parallel over the 16 heads -> 4 heads/core).
c_attn is column-sharded (each core computes K/Q/V features only for its 4
heads); c_proj is row-sharded (each core contracts its 4 heads' attn output
against the matching w_proj columns and emits a full-width partial output).
The 4 partial outputs per batch are summed on the host (+ b_proj).

Per-core device pipeline (all matmuls bf16 with fp32 PSUM accumulation):
  1. KQ^T = (w_kq x)        -> [feat, T] layout, feat on partitions
  2. V    = (x^T w_v^T)     -> [T, d] natural layout, augmented with a
     ones column so the AV matmul also yields the softmax denominators
  3. per head pair, per 512-wide q chunk, over live (causal) k tiles:
       aff^T[k,q] for both heads -> one [128, 2, 512] PSUM pair (2 banks)
       E = exp(0.125*aff^T)      -> ONE wide ScalarE activation per tile
                                    (strided view on diagonal tiles), bf16
       diagonal-tile mask multiply runs on the Pool engine
       [attn^T unnorm; sums] += V_aug.T E   (M=65, per head)
     epilogue: reciprocal rows (DVE), partition_broadcast (Pool),
     normalize (DVE).  No PE involvement in the epilogue.
  4. out_partial = attn^T.T w_proj_slice -> PSUM, DMA'd straight to HBM.

Because each engine executes its compiled instruction stream strictly in
order, emission order is scheduling: aff runs 2 tiles ahead of AV (hides
the ScalarE exp latency), block epilogues are deferred into the next
block, and filler matmuls (K/Q for the other head pair, V tiles, output
projection) are injected mid-block wherever ScalarE would otherwise be
the per-tile rate limiter.
"""

import os
import sys

for _p in ("/opt/trn_rl_repo",):
    if os.path.isdir(_p) and _p not in sys.path:
        sys.path.append(_p)

import numpy as np
import ml_dtypes

B, T, C, H, D = 2, 2048, 1024, 16, 64
N_CORES = 8
HPC = H // 4          # heads per core = 4
CPC = HPC * D         # attn feature cols per core = 256
KQF = 2 * CPC         # K+Q features per core = 512
TCH = 512             # q-chunk width
NJ = T // TCH         # 4 q chunks
NTI = T // 128        # 16 t tiles

_CACHE = {}


def _build_program():
    from contextlib import ExitStack

    import concourse.bass as bass
    import concourse.mybir as mybir
    import concourse.tile as tile
    from concourse import bacc
    from concourse.bass import ts

    f32 = mybir.dt.float32
    bf16 = mybir.dt.bfloat16
    Exp = mybir.ActivationFunctionType.Exp

    nc = bacc.Bacc("TRN2", target_bir_lowering=False, debug=False,
                   num_devices=N_CORES)

    xT_d = nc.dram_tensor("xT", [128, 8, T], bf16, kind="ExternalInput")
    wkq_d = nc.dram_tensor("wkq", [128, 8, KQF], bf16, kind="ExternalInput")
    bkq_d = nc.dram_tensor("bkq", [128, 4], f32, kind="ExternalInput")
    wv_d = nc.dram_tensor("wv", [128, 8, CPC], bf16, kind="ExternalInput")
    wp_d = nc.dram_tensor("wp", [128, 2, C], bf16, kind="ExternalInput")
    mask_d = nc.dram_tensor("mask", [128, 2, 128], bf16, kind="ExternalInput")
    out_d = nc.dram_tensor("out", [T, C], bf16, kind="ExternalOutput")

    with tile.TileContext(nc) as tc, ExitStack() as ctx:
        pp = ctx.enter_context(tc.tile_pool(name="persist", bufs=1))
        xT_sb = pp.tile([128, 8, T], bf16)
        wkq_sb = pp.tile([128, 8, KQF], bf16)
        bkq_sb = pp.tile([128, 4], f32)
        wv_sb = pp.tile([128, 8, CPC], bf16)
        wp_sb = pp.tile([128, 2, C], bf16)
        mask_sb = pp.tile([128, 2, 128], bf16)
        kq_sb = pp.tile([128, 4, T], bf16)
        v_sb = pp.tile([128, NTI, HPC, D + 1], bf16)
        attn_sb = pp.tile([128, 2, T], bf16)

        nc.sync.dma_start(bkq_sb[:], bkq_d[:])
        # critical path first: per c-chunk, just the wkq slices the first
        # two KQ tiles need (m0: K heads 0-1, m2: Q heads 0-1) and the
        # first xT chunk; xT rides a second DMA queue (scalar engine's).
        for c in range(8):
            nc.sync.dma_start(wkq_sb[:, c, 0:128], wkq_d[:, c, 0:128])
            nc.sync.dma_start(wkq_sb[:, c, 256:384], wkq_d[:, c, 256:384])
            nc.scalar.dma_start(xT_sb[:, c, 0:TCH], xT_d[:, c, 0:TCH])
        for c in range(8):
            nc.sync.dma_start(wkq_sb[:, c, 128:256], wkq_d[:, c, 128:256])
            nc.sync.dma_start(wkq_sb[:, c, 384:512], wkq_d[:, c, 384:512])
        nc.sync.dma_start(wv_sb[:], wv_d[:])
        for tch in range(1, NJ):
            for ch in range(2):
                nc.scalar.dma_start(
                    xT_sb[:, ts(ch, 4), ts(tch, TCH)],
                    xT_d[:, ts(ch, 4), ts(tch, TCH)])
        nc.sync.dma_start(wp_sb[:], wp_d[:])
        nc.sync.dma_start(mask_sb[:], mask_d[:])
        for ti in range(NTI):
            nc.any.memset(v_sb[:, ti, :, D:D + 1], 1.0)

        # PSUM: aff pairs 2x[128,2,512] (4 banks) + acc 2x[128,512]
        # (2 banks) + work 2x[128,512] (2 banks) = 8 banks.
        pa_pool = ctx.enter_context(
            tc.tile_pool(name="pall", bufs=1, space="PSUM"))
        e_pool = ctx.enter_context(tc.tile_pool(name="epool", bufs=1))
        r_pool = ctx.enter_context(tc.tile_pool(name="rpool", bufs=1))
        o_pool = ctx.enter_context(tc.tile_pool(name="outp", bufs=1))

        def emit_kq_tile(m, tch):
            pk = pa_pool.tile([128, TCH], f32, tag="work", bufs=2, name="pkq")
            for c in range(8):
                nc.tensor.matmul(
                    pk[:], wkq_sb[:, c, ts(m, 128)],
                    xT_sb[:, c, ts(tch, TCH)],
                    start=(c == 0), stop=(c == 7))
            nc.vector.tensor_scalar_add(
                kq_sb[:, m, ts(tch, TCH)], pk[:], bkq_sb[:, m:m + 1])

        def emit_v(ti):
            pv = pa_pool.tile([128, CPC], f32, tag="work", bufs=2, name="pv")
            for c in range(8):
                nc.tensor.matmul(
                    pv[:], xT_sb[:, c, ts(ti, 128)], wv_sb[:, c, :],
                    start=(c == 0), stop=(c == 7))
            nc.vector.tensor_copy(
                v_sb[:, ti, :, 0:D],
                pv[:].rearrange("p (h d) -> p h d", h=HPC))

        def emit_attn_block(g, j, hooks=(), fillers=(), filler_start=2):
            """Emit one (head-pair, q-chunk) attention block.

            `hooks` run once at tile 1 (used for the previous block's
            deferred epilogue).  `fillers` are closures emitting ~1-2 PE
            matmuls each; they are drained one per tile from tile 4 on, so
            the PE has independent work wherever ScalarE exp would
            otherwise gate the AV matmuls.  Returns the epilogue closure.
            """
            pav0 = pa_pool.tile([128, TCH], f32, tag="acc", bufs=2,
                                name="pav0")
            pav1 = pa_pool.tile([128, TCH], f32, tag="acc", bufs=2,
                                name="pav1")
            n_live = 4 * j + 4
            es = {}
            fillq = list(fillers)

            def emit_aff(i):
                # diagonal tiles only touch queries q >= k: narrow the
                # q-range to [q0:TCH]
                q0 = max(0, 128 * i - TCH * j)
                qsl = slice(j * TCH + q0, (j + 1) * TCH)
                ap = pa_pool.tile([128, 2, TCH], f32, tag="aff", bufs=2,
                                  name="affp")
                nc.tensor.matmul(
                    ap[:, 0, q0:], kq_sb[0:64, g, ts(i, 128)],
                    kq_sb[0:64, 2 + g, qsl], start=True, stop=True)
                nc.tensor.matmul(
                    ap[:, 1, q0:], kq_sb[64:128, g, ts(i, 128)],
                    kq_sb[64:128, 2 + g, qsl], start=True, stop=True)
                ep = e_pool.tile([128, 2, TCH], bf16, tag="e", bufs=4,
                                 name="ep")
                nc.scalar.activation(ep[:, :, q0:], ap[:, :, q0:], Exp,
                                     scale=0.125)
                if q0 > 0 or i == 4 * j:
                    nc.gpsimd.tensor_mul(
                        ep[:, :, q0:q0 + 128], ep[:, :, q0:q0 + 128],
                        mask_sb[:])
                es[i] = (ep, q0)

            def emit_av(i):
                ep, q0 = es.pop(i)
                first, last = (i == 0), (i == n_live - 1)
                nc.tensor.matmul(
                    pav0[0:65, q0:], v_sb[:, i, 2 * g + 0, :],
                    ep[:, 0, q0:], start=first, stop=last)
                nc.tensor.matmul(
                    pav1[0:65, q0:], v_sb[:, i, 2 * g + 1, :],
                    ep[:, 1, q0:], start=first, stop=last)

            for i in range(n_live):
                emit_aff(i)
                if i == 1:
                    for h in hooks:
                        h()
                if i >= 2:
                    emit_av(i - 2)
                if fillq and i >= filler_start:
                    fillq.pop(0)()
            for f in fillq:
                f()
            if n_live >= 2:
                emit_av(n_live - 2)
            emit_av(n_live - 1)

            def finalize():
                rA = r_pool.tile([1, TCH], f32, tag="rA", bufs=2)
                rB = r_pool.tile([1, TCH], f32, tag="rB", bufs=2)
                nc.vector.reciprocal(rA[0:1, :], pav0[64:65, :])
                nc.vector.reciprocal(rB[0:1, :], pav1[64:65, :])
                rbA = r_pool.tile([128, TCH], f32, tag="rbA", bufs=2)
                rbB = r_pool.tile([128, TCH], f32, tag="rbB", bufs=2)
                nc.gpsimd.partition_broadcast(rbA[:], rA[0:1, :],
                                              channels=128)
                nc.gpsimd.partition_broadcast(rbB[:], rB[0:1, :],
                                              channels=128)
                nc.vector.tensor_mul(
                    attn_sb[0:64, g, ts(j, TCH)], pav0[0:64, :],
                    rbA[0:64, :])
                nc.vector.tensor_mul(
                    attn_sb[64:128, g, ts(j, TCH)], pav1[0:64, :],
                    rbB[64:128, :])

            return finalize

        def proj_unit(ti, och):
            def u():
                po = pa_pool.tile([128, 512], f32, tag="work", bufs=2,
                                  name="po")
                nc.tensor.matmul(
                    po[:], attn_sb[:, 0, ts(ti, 128)],
                    wp_sb[:, 0, ts(och, 512)], start=True, stop=False)
                nc.tensor.matmul(
                    po[:], attn_sb[:, 1, ts(ti, 128)],
                    wp_sb[:, 1, ts(och, 512)], start=False, stop=True)
                ot = o_pool.tile([128, 512], bf16, tag="ot", bufs=4)
                nc.vector.tensor_copy(ot[:], po[:])
                nc.sync.dma_start(out_d[ts(ti, 128), ts(och, 512)], ot[:])
            return u

        def proj_units(j):
            return [proj_unit(ti, och)
                    for ti in range(4 * j, 4 * j + 4) for och in range(2)]

        def kq_filler(m, tch):
            return lambda: emit_kq_tile(m, tch)

        def v_filler(ti):
            return lambda: emit_v(ti)

        # loop 1: KQ/V production interleaved with g0 attention blocks.
        fin = None
        for tch in range(NJ):
            emit_kq_tile(0, tch)
            emit_kq_tile(2, tch)
            if tch == 0:
                for ti in range(4):
                    emit_v(ti)
            fillers = []
            if tch < NJ - 1:
                fillers.append(kq_filler(1, tch))
                fillers.append(kq_filler(3, tch))
                fillers.extend(v_filler(ti)
                               for ti in range(4 * tch + 4, 4 * tch + 8))
            else:
                # kq(1,3) is consumed only by late tiles of block (g1,3);
                # it becomes that block's filler instead.
                fillers.append(kq_filler(3, tch))
            hooks = [fin] if fin else []
            fin = emit_attn_block(0, tch, hooks=hooks, fillers=fillers,
                                  filler_start=2)

        # loop 2: g1 attention blocks, largest first so the smallest block
        # (and its ScalarE lag) lands at the kernel tail; projection units
        # of the previously finished chunk serve as PE fillers.
        prev_j = None
        for j in range(NJ - 1, -1, -1):
            hooks = [fin]
            fillers = []
            if j == NJ - 1:
                fillers.append(kq_filler(1, NJ - 1))
            if prev_j is not None:
                fillers.extend(proj_units(prev_j))
            fin = emit_attn_block(1, j, hooks=hooks, fillers=fillers,
                                  filler_start=6)
            prev_j = j
        fin()
        for u in proj_units(0):
            u()

    nc.compile()
    return nc


def _get_program():
    if "nc" not in _CACHE:
        _CACHE["nc"] = _build_program()
    return _CACHE["nc"]


def _host_mask():
    # mask[p, s, c] = 1.0 iff key-local p <= query-local c, duplicated over
    # s (the two heads of a pair share the mask)
    i = np.arange(128)[:, None]
    jj = np.arange(128)[None, :]
    m = (i <= jj).astype(ml_dtypes.bfloat16)
    return np.ascontiguousarray(
        np.broadcast_to(m[:, None, :], (128, 2, 128)))


def _shard_inputs(x, w_attn, b_attn, w_proj, b_proj):
    bf = ml_dtypes.bfloat16
    mask = _host_mask()
    in_maps = []
    for c in range(N_CORES):
        b, g = divmod(c, 4)
        hs = slice(g * CPC, (g + 1) * CPC)
        # xT: (C, T) -> (128, 8, T)
        xT = np.ascontiguousarray(
            x[b].T.reshape(8, 128, T).transpose(1, 0, 2)).astype(bf)
        # K block rows 0:C, Q rows C:2C, V rows 2C:3C of w_attn
        wkq = np.concatenate([w_attn[0 + g * CPC:0 + (g + 1) * CPC],
                              w_attn[C + g * CPC:C + (g + 1) * CPC]], axis=0)
        # (KQF, C) -> transpose -> (C, KQF) -> (128, 8, KQF)
        wkq = np.ascontiguousarray(
            wkq.T.reshape(8, 128, KQF).transpose(1, 0, 2)).astype(bf)
        bkq = np.concatenate([b_attn[0 + g * CPC:0 + (g + 1) * CPC],
                              b_attn[C + g * CPC:C + (g + 1) * CPC]])
        bkq = np.ascontiguousarray(bkq.reshape(4, 128).T).astype(np.float32)
        wv = w_attn[2 * C + g * CPC:2 * C + (g + 1) * CPC]  # (CPC, C)
        wv = np.ascontiguousarray(
            wv.T.reshape(8, 128, CPC).transpose(1, 0, 2)).astype(bf)
        wp = w_proj[:, hs].T  # (CPC, C)
        wp = np.ascontiguousarray(
            wp.reshape(2, 128, C).transpose(1, 0, 2)).astype(bf)
        in_maps.append({"xT": xT, "wkq": wkq, "bkq": bkq, "wv": wv,
                        "wp": wp, "mask": mask})
    return in_maps


def kernel(x, w_attn, b_attn, w_proj, b_proj):
    from concourse.bass_utils import run_bass_kernel_spmd

    nc = _get_program()
    in_maps = _shard_inputs(x, w_attn, b_attn, w_proj, b_proj)
    res = run_bass_kernel_spmd(nc, in_maps, core_ids=list(range(N_CORES)))
    out = np.zeros((B, T, C), dtype=np.float32)
    for c in range(N_CORES):
        b = c // 4
        out[b] += res.results[c]["out"].astype(np.float32)
    # V-bias contribution folded out of the device kernel:
    # (attn + bv)^T @ wp  =  attn^T @ wp  +  (bv @ wp)
    bv_full = b_attn[2 * C:3 * C].astype(np.float64)
    bias_out = bv_full @ w_proj.T.astype(np.float64)
    out += (b_proj.astype(np.float64) + bias_out)[None, None, :].astype(
        np.float32)
    return out


# revision 25
# speedup vs baseline: 1.2284x; 1.2284x over previous
"""Causal multi-head self-attention block (B=2, T=2048, C=1024, H=16) on 8
Trainium2 NeuronCores.

Sharding: core c = 4*b + g handles batch b (2-way data parallel) and head
group g (4-way tensor parallel over the 16 heads -> 4 heads/core).
c_attn is column-sharded (each core computes K/Q/V features only for its 4
heads); c_proj is row-sharded (each core contracts its 4 heads' attn output
against the matching w_proj columns and emits a full-width partial output).
The 4 partial outputs per batch are summed on the host (+ b_proj).

Per-core device pipeline (all matmuls bf16 with fp32 PSUM accumulation):
  1. KQ^T = (w_kq x)        -> [feat, T] layout, feat on partitions
  2. V    = (x^T w_v^T)     -> [T, d] natural layout, augmented with a
     ones column so the AV matmul also yields the softmax denominators
  3. per head pair, per 512-wide q chunk, over live (causal) k tiles:
       aff^T[k,q] for both heads -> one [128, 2, 512] PSUM pair (2 banks)
       E = exp(0.125*aff^T)      -> ONE wide ScalarE activation per tile
                                    (strided view on diagonal tiles), bf16
       diagonal-tile mask multiply runs on the Pool engine
       [attn^T unnorm; sums] += V_aug.T E   (M=65, per head)
     epilogue: reciprocal rows (DVE), partition_broadcast (Pool),
     normalize (DVE).  No PE involvement in the epilogue.
  4. out_partial = attn^T.T w_proj_slice -> PSUM, DMA'd straight to HBM.

Because each engine executes its compiled instruction stream strictly in
order, emission order is scheduling: aff runs 2 tiles ahead of AV (hides
the ScalarE exp latency), block epilogues are deferred into the next
block, and filler matmuls (K/Q for the other head pair, V tiles, output
projection) are injected mid-block wherever ScalarE would otherwise be
the per-tile rate limiter.
"""

import os
import sys

for _p in ("/opt/trn_rl_repo",):
    if os.path.isdir(_p) and _p not in sys.path:
        sys.path.append(_p)

import numpy as np
import ml_dtypes

B, T, C, H, D = 2, 2048, 1024, 16, 64
N_CORES = 8
HPC = H // 4          # heads per core = 4
CPC = HPC * D         # attn feature cols per core = 256
KQF = 2 * CPC         # K+Q features per core = 512
TCH = 512             # q-chunk width
NJ = T // TCH         # 4 q chunks
NTI = T // 128        # 16 t tiles

_CACHE = {}


def _build_program():
    from contextlib import ExitStack

    import concourse.bass as bass
    import concourse.mybir as mybir
    import concourse.tile as tile
    from concourse import bacc
    from concourse.bass import ts

    f32 = mybir.dt.float32
    bf16 = mybir.dt.bfloat16
    Exp = mybir.ActivationFunctionType.Exp

    nc = bacc.Bacc("TRN2", target_bir_lowering=False, debug=False,
                   num_devices=N_CORES)

    xT_d = nc.dram_tensor("xT", [128, 8, T], bf16, kind="ExternalInput")
    wkq_d = nc.dram_tensor("wkq", [128, 8, KQF], bf16, kind="ExternalInput")
    bkq_d = nc.dram_tensor("bkq", [128, 4], f32, kind="ExternalInput")
    wv_d = nc.dram_tensor("wv", [128, 8, CPC], bf16, kind="ExternalInput")
    wp_d = nc.dram_tensor("wp", [128, 2, C], bf16, kind="ExternalInput")
    mask_d = nc.dram_tensor("mask", [128, 2, 128], bf16, kind="ExternalInput")
    out_d = nc.dram_tensor("out", [T, C], bf16, kind="ExternalOutput")

    with tile.TileContext(nc) as tc, ExitStack() as ctx:
        pp = ctx.enter_context(tc.tile_pool(name="persist", bufs=1))
        xT_sb = pp.tile([128, 8, T], bf16)
        wkq_sb = pp.tile([128, 8, KQF], bf16)
        bkq_sb = pp.tile([128, 4], f32)
        wv_sb = pp.tile([128, 8, CPC], bf16)
        wp_sb = pp.tile([128, 2, C], bf16)
        mask_sb = pp.tile([128, 2, 128], bf16)
        kq_sb = pp.tile([128, 4, T], bf16)
        v_sb = pp.tile([128, NTI, HPC, D + 1], bf16)
        attn_sb = pp.tile([128, 2, T], bf16)

        # critical path first.  Host orders wkq features [K01, Q01, K23,
        # Q23], so the half the first head-pair needs is one strided DMA;
        # xT's first chunk goes per-c so KQ matmuls unlock progressively.
        nc.sync.dma_start(wkq_sb[:, 0, 0:256], wkq_d[:, 0, 0:256])
        nc.sync.dma_start(xT_sb[:, 0, 0:TCH], xT_d[:, 0, 0:TCH])
        nc.sync.dma_start(bkq_sb[:], bkq_d[:])
        nc.sync.dma_start(wkq_sb[:, 1:8, 0:256], wkq_d[:, 1:8, 0:256])
        for c in range(1, 8):
            nc.sync.dma_start(xT_sb[:, c, 0:TCH], xT_d[:, c, 0:TCH])
        nc.sync.dma_start(wv_sb[:], wv_d[:])
        nc.sync.dma_start(wkq_sb[:, :, 256:512], wkq_d[:, :, 256:512])
        for tch in range(1, NJ):
            nc.sync.dma_start(xT_sb[:, :, ts(tch, TCH)],
                              xT_d[:, :, ts(tch, TCH)])
        nc.sync.dma_start(wp_sb[:], wp_d[:])
        nc.sync.dma_start(mask_sb[:], mask_d[:])
        for ti in range(NTI):
            nc.any.memset(v_sb[:, ti, :, D:D + 1], 1.0)

        # PSUM: aff pairs 2x[128,2,512] (4 banks) + acc 2x[128,512]
        # (2 banks) + work 2x[128,512] (2 banks) = 8 banks.
        pa_pool = ctx.enter_context(
            tc.tile_pool(name="pall", bufs=1, space="PSUM"))
        e_pool = ctx.enter_context(tc.tile_pool(name="epool", bufs=1))
        r_pool = ctx.enter_context(tc.tile_pool(name="rpool", bufs=1))
        o_pool = ctx.enter_context(tc.tile_pool(name="outp", bufs=1))

        def emit_kq_tiles(ms, tch):
            # interleave the c-loops of several feature tiles so the PE can
            # advance as each 128-row chunk of x arrives from HBM
            pk = {m: pa_pool.tile([128, TCH], f32, tag="work", bufs=2,
                                  name="pkq") for m in ms}
            for c in range(8):
                for m in ms:
                    nc.tensor.matmul(
                        pk[m][:], wkq_sb[:, c, ts(m, 128)],
                        xT_sb[:, c, ts(tch, TCH)],
                        start=(c == 0), stop=(c == 7))
            for m in ms:
                nc.vector.tensor_scalar_add(
                    kq_sb[:, m, ts(tch, TCH)], pk[m][:], bkq_sb[:, m:m + 1])

        def emit_kq_tile(m, tch):
            emit_kq_tiles([m], tch)

        def emit_v(ti):
            pv = pa_pool.tile([128, CPC], f32, tag="work", bufs=2, name="pv")
            for c in range(8):
                nc.tensor.matmul(
                    pv[:], xT_sb[:, c, ts(ti, 128)], wv_sb[:, c, :],
                    start=(c == 0), stop=(c == 7))
            nc.vector.tensor_copy(
                v_sb[:, ti, :, 0:D],
                pv[:].rearrange("p (h d) -> p h d", h=HPC))

        def emit_attn_block(g, j, hooks=(), fillers=(), filler_start=2):
            """Emit one (head-pair, q-chunk) attention block.

            `hooks` run once at tile 1 (used for the previous block's
            deferred epilogue).  `fillers` are closures emitting ~1-2 PE
            matmuls each; they are drained one per tile from tile 4 on, so
            the PE has independent work wherever ScalarE exp would
            otherwise gate the AV matmuls.  Returns the epilogue closure.
            """
            pav0 = pa_pool.tile([128, TCH], f32, tag="acc", bufs=2,
                                name="pav0")
            pav1 = pa_pool.tile([128, TCH], f32, tag="acc", bufs=2,
                                name="pav1")
            n_live = 4 * j + 4
            es = {}
            fillq = list(fillers)

            def emit_aff(i):
                # diagonal tiles only touch queries q >= k: narrow the
                # q-range to [q0:TCH]
                q0 = max(0, 128 * i - TCH * j)
                qsl = slice(j * TCH + q0, (j + 1) * TCH)
                ap = pa_pool.tile([128, 2, TCH], f32, tag="aff", bufs=2,
                                  name="affp")
                nc.tensor.matmul(
                    ap[:, 0, q0:], kq_sb[0:64, 2 * g, ts(i, 128)],
                    kq_sb[0:64, 2 * g + 1, qsl], start=True, stop=True)
                nc.tensor.matmul(
                    ap[:, 1, q0:], kq_sb[64:128, 2 * g, ts(i, 128)],
                    kq_sb[64:128, 2 * g + 1, qsl], start=True, stop=True)
                ep = e_pool.tile([128, 2, TCH], bf16, tag="e", bufs=6,
                                 name="ep")
                nc.scalar.activation(ep[:, :, q0:], ap[:, :, q0:], Exp,
                                     scale=0.125)
                if q0 > 0 or i == 4 * j:
                    nc.gpsimd.tensor_mul(
                        ep[:, :, q0:q0 + 128], ep[:, :, q0:q0 + 128],
                        mask_sb[:])
                es[i] = (ep, q0)

            def emit_av(i):
                ep, q0 = es.pop(i)
                first, last = (i == 0), (i == n_live - 1)
                nc.tensor.matmul(
                    pav0[0:65, q0:], v_sb[:, i, 2 * g + 0, :],
                    ep[:, 0, q0:], start=first, stop=last)
                nc.tensor.matmul(
                    pav1[0:65, q0:], v_sb[:, i, 2 * g + 1, :],
                    ep[:, 1, q0:], start=first, stop=last)

            look = min(4, n_live)
            for i in range(n_live):
                emit_aff(i)
                if i == 1:
                    for h in hooks:
                        h()
                if i >= look:
                    emit_av(i - look)
                if fillq and i >= filler_start:
                    fillq.pop(0)()
            for i in range(n_live - look, n_live):
                emit_av(i)
                if fillq:
                    fillq.pop(0)()
            for f in fillq:
                f()

            def finalize():
                rA = r_pool.tile([1, TCH], f32, tag="rA", bufs=2)
                rB = r_pool.tile([1, TCH], f32, tag="rB", bufs=2)
                nc.vector.reciprocal(rA[0:1, :], pav0[64:65, :])
                nc.vector.reciprocal(rB[0:1, :], pav1[64:65, :])
                rbA = r_pool.tile([128, TCH], f32, tag="rbA", bufs=2)
                rbB = r_pool.tile([128, TCH], f32, tag="rbB", bufs=2)
                nc.gpsimd.partition_broadcast(rbA[:], rA[0:1, :],
                                              channels=128)
                nc.gpsimd.partition_broadcast(rbB[:], rB[0:1, :],
                                              channels=128)
                nc.vector.tensor_mul(
                    attn_sb[0:64, g, ts(j, TCH)], pav0[0:64, :],
                    rbA[0:64, :])
                nc.vector.tensor_mul(
                    attn_sb[64:128, g, ts(j, TCH)], pav1[0:64, :],
                    rbB[64:128, :])

            return finalize

        def proj_unit(ti, och):
            def u():
                po = pa_pool.tile([128, 512], f32, tag="work", bufs=2,
                                  name="po")
                nc.tensor.matmul(
                    po[:], attn_sb[:, 0, ts(ti, 128)],
                    wp_sb[:, 0, ts(och, 512)], start=True, stop=False)
                nc.tensor.matmul(
                    po[:], attn_sb[:, 1, ts(ti, 128)],
                    wp_sb[:, 1, ts(och, 512)], start=False, stop=True)
                ot = o_pool.tile([128, 512], bf16, tag="ot", bufs=4)
                nc.vector.tensor_copy(ot[:], po[:])
                nc.sync.dma_start(out_d[ts(ti, 128), ts(och, 512)], ot[:])
            return u

        def proj_units(j):
            return [proj_unit(ti, och)
                    for ti in range(4 * j, 4 * j + 4) for och in range(2)]

        def kq_filler(m, tch):
            return lambda: emit_kq_tile(m, tch)

        def v_filler(ti):
            return lambda: emit_v(ti)

        # loop 1: KQ/V production interleaved with g0 attention blocks.
        # Feature-tile order is [K01, Q01, K23, Q23]: g0 needs m0/m1; g1's
        # m2/m3 chunks are deferred into loop 2 as just-in-time fillers
        # (only chunk 0 must exist before block (g1,0) starts).
        fin = None
        for tch in range(NJ):
            emit_kq_tiles([0, 1], tch)
            if tch == 0:
                for ti in range(4):
                    emit_v(ti)
            fillers = []
            if tch < NJ - 1:
                fillers.extend(v_filler(ti)
                               for ti in range(4 * tch + 4, 4 * tch + 8))
            else:
                fillers.append(kq_filler(2, 0))
                fillers.append(kq_filler(3, 0))
            hooks = [fin] if fin else []
            fin = emit_attn_block(0, tch, hooks=hooks, fillers=fillers,
                                  filler_start=2)

        # loop 2: g1 attention blocks ascending; each block's fillers are
        # the NEXT chunk's K23/Q23 tiles plus the projection units of the
        # previously finalized chunk.
        prev_j = None
        for j in range(NJ):
            hooks = [fin]
            fillers = []
            if j + 1 < NJ:
                fillers.append(kq_filler(2, j + 1))
                fillers.append(kq_filler(3, j + 1))
            if prev_j is not None:
                fillers.extend(proj_units(prev_j))
            fin = emit_attn_block(1, j, hooks=hooks, fillers=fillers,
                                  filler_start=4)
            prev_j = j
        fin()
        for u in proj_units(NJ - 1):
            u()

    nc.compile()
    return nc


def _get_program():
    if "nc" not in _CACHE:
        _CACHE["nc"] = _build_program()
    return _CACHE["nc"]


def _host_mask():
    # mask[p, s, c] = 1.0 iff key-local p <= query-local c, duplicated over
    # s (the two heads of a pair share the mask)
    i = np.arange(128)[:, None]
    jj = np.arange(128)[None, :]
    m = (i <= jj).astype(ml_dtypes.bfloat16)
    return np.ascontiguousarray(
        np.broadcast_to(m[:, None, :], (128, 2, 128)))


def _shard_inputs(x, w_attn, b_attn, w_proj, b_proj):
    bf = ml_dtypes.bfloat16
    mask = _host_mask()
    in_maps = []
    for c in range(N_CORES):
        b, g = divmod(c, 4)
        hs = slice(g * CPC, (g + 1) * CPC)
        # xT: (C, T) -> (128, 8, T)
        xT = np.ascontiguousarray(
            x[b].T.reshape(8, 128, T).transpose(1, 0, 2)).astype(bf)
        # K block rows 0:C, Q rows C:2C, V rows 2C:3C of w_attn.  Feature
        # tiles ordered [K01, Q01, K23, Q23] so the first head pair's
        # K and Q are one contiguous half.
        Kr = w_attn[g * CPC:(g + 1) * CPC]
        Qr = w_attn[C + g * CPC:C + (g + 1) * CPC]
        wkq = np.concatenate([Kr[0:128], Qr[0:128],
                              Kr[128:256], Qr[128:256]], axis=0)
        # (KQF, C) -> transpose -> (C, KQF) -> (128, 8, KQF)
        wkq = np.ascontiguousarray(
            wkq.T.reshape(8, 128, KQF).transpose(1, 0, 2)).astype(bf)
        bK = b_attn[g * CPC:(g + 1) * CPC]
        bQ = b_attn[C + g * CPC:C + (g + 1) * CPC]
        bkq = np.concatenate([bK[0:128], bQ[0:128], bK[128:256],
                              bQ[128:256]])
        bkq = np.ascontiguousarray(bkq.reshape(4, 128).T).astype(np.float32)
        wv = w_attn[2 * C + g * CPC:2 * C + (g + 1) * CPC]  # (CPC, C)
        wv = np.ascontiguousarray(
            wv.T.reshape(8, 128, CPC).transpose(1, 0, 2)).astype(bf)
        wp = w_proj[:, hs].T  # (CPC, C)
        wp = np.ascontiguousarray(
            wp.reshape(2, 128, C).transpose(1, 0, 2)).astype(bf)
        in_maps.append({"xT": xT, "wkq": wkq, "bkq": bkq, "wv": wv,
                        "wp": wp, "mask": mask})
    return in_maps


def kernel(x, w_attn, b_attn, w_proj, b_proj):
    from concourse.bass_utils import run_bass_kernel_spmd

    nc = _get_program()
    in_maps = _shard_inputs(x, w_attn, b_attn, w_proj, b_proj)
    res = run_bass_kernel_spmd(nc, in_maps, core_ids=list(range(N_CORES)))
    out = np.zeros((B, T, C), dtype=np.float32)
    for c in range(N_CORES):
        b = c // 4
        out[b] += res.results[c]["out"].astype(np.float32)
    # V-bias contribution folded out of the device kernel:
    # (attn + bv)^T @ wp  =  attn^T @ wp  +  (bv @ wp)
    bv_full = b_attn[2 * C:3 * C].astype(np.float64)
    bias_out = bv_full @ w_proj.T.astype(np.float64)
    out += (b_proj.astype(np.float64) + bias_out)[None, None, :].astype(
        np.float32)
    return out


# revision 37
# speedup vs baseline: 1.2707x; 1.0344x over previous
"""Causal multi-head self-attention block (B=2, T=2048, C=1024, H=16) on 8
Trainium2 NeuronCores.

Sharding: core c = 4*b + g handles batch b (2-way data parallel) and head
group g (4-way tensor parallel over the 16 heads -> 4 heads/core).
c_attn is column-sharded (each core computes K/Q/V features only for its 4
heads); c_proj is row-sharded (each core contracts its 4 heads' attn output
against the matching w_proj columns and emits a full-width partial output).
The 4 partial outputs per batch are summed on the host (+ b_proj).

Per-core device pipeline (all matmuls bf16 with fp32 PSUM accumulation):
  1. KQ^T = (w_kq x)        -> [feat, T] layout, feat on partitions
  2. V    = (x^T w_v^T)     -> [T, d] natural layout, augmented with a
     ones column so the AV matmul also yields the softmax denominators
  3. per head pair, per 512-wide q chunk, over live (causal) k tiles:
       aff^T[k,q] for both heads -> one [128, 2, 512] PSUM pair (2 banks)
       E = exp(0.125*aff^T)      -> ONE wide ScalarE activation per tile
                                    (strided view on diagonal tiles), bf16
       diagonal-tile mask multiply runs on the Pool engine
       [attn^T unnorm; sums] += V_aug.T E   (M=65, per head)
     epilogue: reciprocal rows (DVE), partition_broadcast (Pool),
     normalize (DVE).  No PE involvement in the epilogue.
  4. out_partial = attn^T.T w_proj_slice -> PSUM, DMA'd straight to HBM.

Because each engine executes its compiled instruction stream strictly in
order, emission order is scheduling: aff runs 2 tiles ahead of AV (hides
the ScalarE exp latency), block epilogues are deferred into the next
block, and filler matmuls (K/Q for the other head pair, V tiles, output
projection) are injected mid-block wherever ScalarE would otherwise be
the per-tile rate limiter.
"""

import os
import sys

for _p in ("/opt/trn_rl_repo",):
    if os.path.isdir(_p) and _p not in sys.path:
        sys.path.append(_p)

import numpy as np
import ml_dtypes

B, T, C, H, D = 2, 2048, 1024, 16, 64
N_CORES = 8
HPC = H // 4          # heads per core = 4
CPC = HPC * D         # attn feature cols per core = 256
KQF = 2 * CPC         # K+Q features per core = 512
TCH = 512             # q-chunk width
NJ = T // TCH         # 4 q chunks
NTI = T // 128        # 16 t tiles

_CACHE = {}


def _build_program():
    from contextlib import ExitStack

    import concourse.bass as bass
    import concourse.mybir as mybir
    import concourse.tile as tile
    from concourse import bacc
    from concourse.bass import ts

    f32 = mybir.dt.float32
    bf16 = mybir.dt.bfloat16
    Exp = mybir.ActivationFunctionType.Exp

    nc = bacc.Bacc("TRN2", target_bir_lowering=False, debug=False,
                   num_devices=N_CORES)

    xq0_d = nc.dram_tensor("xq0", [128, 8, 256 + TCH], bf16,
                           kind="ExternalInput")
    xT_d = nc.dram_tensor("xT", [128, 8, T], bf16, kind="ExternalInput")
    wkq_d = nc.dram_tensor("wkq", [128, 8, KQF], bf16, kind="ExternalInput")
    bkq_d = nc.dram_tensor("bkq", [128, 4], f32, kind="ExternalInput")
    wv_d = nc.dram_tensor("wv", [128, 8, CPC], bf16, kind="ExternalInput")
    wp_d = nc.dram_tensor("wp", [128, 2, C], bf16, kind="ExternalInput")
    mask_d = nc.dram_tensor("mask", [128, 2, 128], bf16, kind="ExternalInput")
    out_d = nc.dram_tensor("out", [T, C], bf16, kind="ExternalOutput")

    with tile.TileContext(nc) as tc, ExitStack() as ctx:
        pp = ctx.enter_context(tc.tile_pool(name="persist", bufs=1))
        xq0_sb = pp.tile([128, 8, 256 + TCH], bf16)
        xT_sb = pp.tile([128, 8, T], bf16)
        wkq_sb = pp.tile([128, 8, KQF], bf16)
        bkq_sb = pp.tile([128, 4], f32)
        wv_sb = pp.tile([128, 8, CPC], bf16)
        wp_sb = pp.tile([128, 2, C], bf16)
        mask_sb = pp.tile([128, 2, 128], bf16)
        kq_sb = pp.tile([128, 4, T], bf16)
        v_sb = pp.tile([128, NTI, HPC, D + 1], bf16)
        attn_sb = pp.tile([128, 2, T], bf16)

        # critical path first.  Host orders wkq features [K01, Q01, K23,
        # Q23], so the half the first head-pair needs is one strided DMA;
        # xT's first chunk goes per-c so KQ matmuls unlock progressively.
        # weights ride the Pool engine's SWDGE queue so their descriptor
        # generation runs in parallel with the xT stream's on HWDGE
        for c in range(8):
            nc.sync.dma_start(xq0_sb[:, c, :], xq0_d[:, c, :])
            if c == 0:
                nc.sync.dma_start(bkq_sb[:], bkq_d[:])
        nc.sync.dma_start(wv_sb[:], wv_d[:])
        nc.sync.dma_start(wkq_sb[:, :, 256:512], wkq_d[:, :, 256:512])
        for tch in range(1, NJ):
            nc.sync.dma_start(xT_sb[:, :, ts(tch, TCH)],
                              xT_d[:, :, ts(tch, TCH)])
        nc.sync.dma_start(wp_sb[:], wp_d[:])
        nc.sync.dma_start(mask_sb[:], mask_d[:])
        for ti in range(NTI):
            nc.any.memset(v_sb[:, ti, :, D:D + 1], 1.0)

        # PSUM: aff pairs 2x[128,2,512] (4 banks) + acc 2x[128,512]
        # (2 banks) + work 2x[128,512] (2 banks) = 8 banks.
        pa_pool = ctx.enter_context(
            tc.tile_pool(name="pall", bufs=1, space="PSUM"))
        e_pool = ctx.enter_context(tc.tile_pool(name="epool", bufs=1))
        r_pool = ctx.enter_context(tc.tile_pool(name="rpool", bufs=1))
        o_pool = ctx.enter_context(tc.tile_pool(name="outp", bufs=1))

        def wkq_at(c, m):
            # feature tiles m0/m1 arrive packed with xT chunk 0 in xq0
            if m < 2:
                return xq0_sb[:, c, ts(m, 128)]
            return wkq_sb[:, c, ts(m, 128)]

        def xT_at(c, t0, t1):
            if t1 <= TCH:
                return xq0_sb[:, c, 256 + t0:256 + t1]
            return xT_sb[:, c, t0:t1]

        def emit_kq_tiles(ms, tch):
            # interleave the c-loops of several feature tiles so the PE can
            # advance as each 128-row chunk of x arrives from HBM
            pk = {m: pa_pool.tile([128, TCH], f32, tag="work", bufs=2,
                                  name="pkq") for m in ms}
            for c in range(8):
                for m in ms:
                    nc.tensor.matmul(
                        pk[m][:], wkq_at(c, m),
                        xT_at(c, tch * TCH, (tch + 1) * TCH),
                        start=(c == 0), stop=(c == 7))
            for m in ms:
                nc.vector.tensor_scalar_add(
                    kq_sb[:, m, ts(tch, TCH)], pk[m][:], bkq_sb[:, m:m + 1])

        def emit_kq_tile(m, tch):
            emit_kq_tiles([m], tch)

        def emit_v(ti):
            pv = pa_pool.tile([128, CPC], f32, tag="work", bufs=2, name="pv")
            for c in range(8):
                nc.tensor.matmul(
                    pv[:], xT_at(c, ti * 128, (ti + 1) * 128), wv_sb[:, c, :],
                    start=(c == 0), stop=(c == 7))
            nc.vector.tensor_copy(
                v_sb[:, ti, :, 0:D],
                pv[:].rearrange("p (h d) -> p h d", h=HPC))

        def emit_attn_block(g, j, hooks=(), fillers=(), filler_start=2):
            """Emit one (head-pair, q-chunk) attention block.

            `hooks` run once at tile 1 (used for the previous block's
            deferred epilogue).  `fillers` are closures emitting ~1-2 PE
            matmuls each; they are drained one per tile from tile 4 on, so
            the PE has independent work wherever ScalarE exp would
            otherwise gate the AV matmuls.  Returns the epilogue closure.
            """
            pav0 = pa_pool.tile([128, TCH], f32, tag="acc", bufs=2,
                                name="pav0")
            pav1 = pa_pool.tile([128, TCH], f32, tag="acc", bufs=2,
                                name="pav1")
            n_live = 4 * j + 4
            es = {}
            fillq = list(fillers)

            def emit_aff(i):
                # diagonal tiles only touch queries q >= k: narrow the
                # q-range to [q0:TCH]
                q0 = max(0, 128 * i - TCH * j)
                qsl = slice(j * TCH + q0, (j + 1) * TCH)
                ap = pa_pool.tile([128, 2, TCH], f32, tag="aff", bufs=2,
                                  name="affp")
                nc.tensor.matmul(
                    ap[:, 0, q0:], kq_sb[0:64, 2 * g, ts(i, 128)],
                    kq_sb[0:64, 2 * g + 1, qsl], start=True, stop=True)
                nc.tensor.matmul(
                    ap[:, 1, q0:], kq_sb[64:128, 2 * g, ts(i, 128)],
                    kq_sb[64:128, 2 * g + 1, qsl], start=True, stop=True)
                ep = e_pool.tile([128, 2, TCH], bf16, tag="e", bufs=6,
                                 name="ep")
                nc.scalar.activation(ep[:, :, q0:], ap[:, :, q0:], Exp,
                                     scale=0.125)
                if q0 > 0 or i == 4 * j:
                    nc.gpsimd.tensor_mul(
                        ep[:, :, q0:q0 + 128], ep[:, :, q0:q0 + 128],
                        mask_sb[:])
                es[i] = (ep, q0)

            def emit_av(i):
                ep, q0 = es.pop(i)
                first, last = (i == 0), (i == n_live - 1)
                nc.tensor.matmul(
                    pav0[0:65, q0:], v_sb[:, i, 2 * g + 0, :],
                    ep[:, 0, q0:], start=first, stop=last)
                nc.tensor.matmul(
                    pav1[0:65, q0:], v_sb[:, i, 2 * g + 1, :],
                    ep[:, 1, q0:], start=first, stop=last)

            look = min(4, n_live)
            for i in range(n_live):
                emit_aff(i)
                if i == 1:
                    for h in hooks:
                        h()
                if i >= look:
                    emit_av(i - look)
                if fillq and i >= filler_start:
                    fillq.pop(0)()
            for i in range(n_live - look, n_live):
                emit_av(i)
                if fillq:
                    fillq.pop(0)()
            for f in fillq:
                f()

            def finalize_cols(sl):
                r2 = r_pool.tile([1, 2, TCH], f32, tag="r2", bufs=2)
                nc.vector.reciprocal(r2[0:1, 0, sl], pav0[64:65, sl])
                nc.vector.reciprocal(r2[0:1, 1, sl], pav1[64:65, sl])
                rb2 = r_pool.tile([128, 2, TCH], f32, tag="rb2", bufs=2)
                nc.gpsimd.partition_broadcast(rb2[:, :, sl], r2[0:1, :, sl],
                                              channels=128)
                qsl = slice(j * TCH + sl.start, j * TCH + sl.stop)
                nc.vector.tensor_mul(
                    attn_sb[0:64, g, qsl], pav0[0:64, sl], rb2[0:64, 0, sl])
                nc.vector.tensor_mul(
                    attn_sb[64:128, g, qsl], pav1[0:64, sl],
                    rb2[64:128, 1, sl])

            def finalize():
                finalize_cols(slice(0, TCH))

            finalize.cols = finalize_cols
            return finalize

        def proj_units(j):
            # (ti, och) units; the two och halves share one [128,1024] SBUF
            # staging tile so each ti goes out as a single DMA.
            units = []
            for ti in range(4 * j, 4 * j + 4):
                ots = {}

                def u(ti=ti, och=0, ots=ots):
                    if och == 0:
                        ots[0] = o_pool.tile([128, C], bf16, tag="ot",
                                             bufs=3, name="ot")
                    po = pa_pool.tile([128, 512], f32, tag="work", bufs=2,
                                      name="po")
                    nc.tensor.matmul(
                        po[:], attn_sb[:, 0, ts(ti, 128)],
                        wp_sb[:, 0, ts(och, 512)], start=True, stop=False)
                    nc.tensor.matmul(
                        po[:], attn_sb[:, 1, ts(ti, 128)],
                        wp_sb[:, 1, ts(och, 512)], start=False, stop=True)
                    nc.vector.tensor_copy(ots[0][:, ts(och, 512)], po[:])
                    if och == 1:
                        nc.sync.dma_start(out_d[ts(ti, 128), :], ots[0][:])
                for och in range(2):
                    units.append(
                        (lambda ti=ti, och=och, ots=ots:
                         u(ti=ti, och=och, ots=ots)))
            return units

        def kq_filler(m, tch):
            return lambda: emit_kq_tile(m, tch)

        def v_filler(ti):
            return lambda: emit_v(ti)

        # loop 1: KQ/V production interleaved with g0 attention blocks.
        # Feature-tile order is [K01, Q01, K23, Q23]: g0 needs m0/m1; g1's
        # m2/m3 chunks are deferred into loop 2 as just-in-time fillers
        # (only chunk 0 must exist before block (g1,0) starts).
        fin = None
        emit_kq_tiles([0, 1], 0)
        for tch in range(NJ):
            if tch == 0:
                for ti in range(4):
                    emit_v(ti)
            fillers = []
            if tch < NJ - 1:
                fillers.append(kq_filler(0, tch + 1))
                fillers.append(kq_filler(1, tch + 1))
                fillers.extend(v_filler(ti)
                               for ti in range(4 * tch + 4, 4 * tch + 8))
            else:
                fillers.append(kq_filler(2, 0))
                fillers.append(kq_filler(3, 0))
            hooks = [fin] if fin else []
            fin = emit_attn_block(0, tch, hooks=hooks, fillers=fillers,
                                  filler_start=2)

        # loop 2: g1 attention blocks ascending; each block's fillers are
        # the NEXT chunk's K23/Q23 tiles plus the projection units of the
        # previously finalized chunk.
        prev_j = None
        for j in range(NJ):
            hooks = [fin]
            fillers = []
            if j + 1 < NJ:
                fillers.append(kq_filler(2, j + 1))
                fillers.append(kq_filler(3, j + 1))
            if prev_j is not None:
                fillers.extend(proj_units(prev_j))
            # blocks whose first fillers are proj units (which wait on the
            # hoisted epilogue's DVE/Pool chain) start filling later
            fin = emit_attn_block(1, j, hooks=hooks, fillers=fillers,
                                  filler_start=2 if j + 1 < NJ else 4)
            prev_j = j
        # pipelined tail: normalize the last chunk 128 columns at a time,
        # launching each t-tile's projection as soon as its piece lands
        units = proj_units(NJ - 1)
        for qb in range(4):
            fin.cols(slice(qb * 128, (qb + 1) * 128))
            units[2 * qb]()
            units[2 * qb + 1]()

    nc.compile()
    return nc


def _get_program():
    if "nc" not in _CACHE:
        _CACHE["nc"] = _build_program()
    return _CACHE["nc"]


def _host_mask():
    # mask[p, s, c] = 1.0 iff key-local p <= query-local c, duplicated over
    # s (the two heads of a pair share the mask)
    i = np.arange(128)[:, None]
    jj = np.arange(128)[None, :]
    m = (i <= jj).astype(ml_dtypes.bfloat16)
    return np.ascontiguousarray(
        np.broadcast_to(m[:, None, :], (128, 2, 128)))


def _shard_inputs(x, w_attn, b_attn, w_proj, b_proj):
    bf = ml_dtypes.bfloat16
    mask = _host_mask()
    in_maps = []
    for c in range(N_CORES):
        b, g = divmod(c, 4)
        hs = slice(g * CPC, (g + 1) * CPC)
        # xT: (C, T) -> (128, 8, T)
        xT = np.ascontiguousarray(
            x[b].T.reshape(8, 128, T).transpose(1, 0, 2)).astype(bf)
        # K block rows 0:C, Q rows C:2C, V rows 2C:3C of w_attn.  Feature
        # tiles ordered [K01, Q01, K23, Q23] so the first head pair's
        # K and Q are one contiguous half.
        Kr = w_attn[g * CPC:(g + 1) * CPC]
        Qr = w_attn[C + g * CPC:C + (g + 1) * CPC]
        wkq = np.concatenate([Kr[0:128], Qr[0:128],
                              Kr[128:256], Qr[128:256]], axis=0)
        # (KQF, C) -> transpose -> (C, KQF) -> (128, 8, KQF)
        wkq = np.ascontiguousarray(
            wkq.T.reshape(8, 128, KQF).transpose(1, 0, 2)).astype(bf)
        bK = b_attn[g * CPC:(g + 1) * CPC]
        bQ = b_attn[C + g * CPC:C + (g + 1) * CPC]
        bkq = np.concatenate([bK[0:128], bQ[0:128], bK[128:256],
                              bQ[128:256]])
        bkq = np.ascontiguousarray(bkq.reshape(4, 128).T).astype(np.float32)
        wv = w_attn[2 * C + g * CPC:2 * C + (g + 1) * CPC]  # (CPC, C)
        wv = np.ascontiguousarray(
            wv.T.reshape(8, 128, CPC).transpose(1, 0, 2)).astype(bf)
        wp = w_proj[:, hs].T  # (CPC, C)
        wp = np.ascontiguousarray(
            wp.reshape(2, 128, C).transpose(1, 0, 2)).astype(bf)
        xq0 = np.ascontiguousarray(
            np.concatenate([wkq[:, :, 0:256], xT[:, :, 0:TCH]], axis=2))
        in_maps.append({"xq0": xq0, "xT": xT, "wkq": wkq, "bkq": bkq,
                        "wv": wv, "wp": wp, "mask": mask})
    return in_maps


def kernel(x, w_attn, b_attn, w_proj, b_proj):
    from concourse.bass_utils import run_bass_kernel_spmd

    nc = _get_program()
    in_maps = _shard_inputs(x, w_attn, b_attn, w_proj, b_proj)
    res = run_bass_kernel_spmd(nc, in_maps, core_ids=list(range(N_CORES)))
    out = np.zeros((B, T, C), dtype=np.float32)
    for c in range(N_CORES):
        b = c // 4
        out[b] += res.results[c]["out"].astype(np.float32)
    # V-bias contribution folded out of the device kernel:
    # (attn + bv)^T @ wp  =  attn^T @ wp  +  (bv @ wp)
    bv_full = b_attn[2 * C:3 * C].astype(np.float64)
    bias_out = bv_full @ w_proj.T.astype(np.float64)
    out += (b_proj.astype(np.float64) + bias_out)[None, None, :].astype(
        np.float32)
    return out


# revision 45
# speedup vs baseline: 1.2883x; 1.0139x over previous
"""Causal multi-head self-attention block (B=2, T=2048, C=1024, H=16) on 8
Trainium2 NeuronCores.

Sharding: core c = 4*b + g handles batch b (2-way data parallel) and head
group g (4-way tensor parallel over the 16 heads -> 4 heads/core).
c_attn is column-sharded (each core computes K/Q/V features only for its 4
heads); c_proj is row-sharded (each core contracts its 4 heads' attn output
against the matching w_proj columns and emits a full-width partial output).
The 4 partial outputs per batch are summed on the host (+ b_proj).

Per-core device pipeline (all matmuls bf16 with fp32 PSUM accumulation):
  1. KQ^T = (w_kq x)        -> [feat, T] layout, feat on partitions
  2. V    = (x^T w_v^T)     -> [T, d] natural layout, augmented with a
     ones column so the AV matmul also yields the softmax denominators
  3. per head pair, per 512-wide q chunk, over live (causal) k tiles:
       aff^T[k,q] for both heads -> one [128, 2, 512] PSUM pair (2 banks)
       E = exp(0.125*aff^T)      -> ONE wide ScalarE activation per tile
                                    (strided view on diagonal tiles), bf16
       diagonal-tile mask multiply runs on the Pool engine
       [attn^T unnorm; sums] += V_aug.T E   (M=65, per head)
     epilogue: reciprocal rows (DVE), partition_broadcast (Pool),
     normalize (DVE).  No PE involvement in the epilogue.
  4. out_partial = attn^T.T w_proj_slice -> PSUM, DMA'd straight to HBM.

Because each engine executes its compiled instruction stream strictly in
order, emission order is scheduling: aff runs 2 tiles ahead of AV (hides
the ScalarE exp latency), block epilogues are deferred into the next
block, and filler matmuls (K/Q for the other head pair, V tiles, output
projection) are injected mid-block wherever ScalarE would otherwise be
the per-tile rate limiter.
"""

import os
import sys

for _p in ("/opt/trn_rl_repo",):
    if os.path.isdir(_p) and _p not in sys.path:
        sys.path.append(_p)

import numpy as np
import ml_dtypes

B, T, C, H, D = 2, 2048, 1024, 16, 64
N_CORES = 8
HPC = H // 4          # heads per core = 4
CPC = HPC * D         # attn feature cols per core = 256
KQF = 2 * CPC         # K+Q features per core = 512
TCH = 512             # q-chunk width
NJ = T // TCH         # 4 q chunks
NTI = T // 128        # 16 t tiles

_CACHE = {}


def _build_program():
    from contextlib import ExitStack

    import concourse.bass as bass
    import concourse.mybir as mybir
    import concourse.tile as tile
    from concourse import bacc
    from concourse.bass import ts

    f32 = mybir.dt.float32
    bf16 = mybir.dt.bfloat16
    Exp = mybir.ActivationFunctionType.Exp

    nc = bacc.Bacc("TRN2", target_bir_lowering=False, debug=False,
                   num_devices=N_CORES)

    xq0_d = nc.dram_tensor("xq0", [128, 8, 256 + TCH], bf16,
                           kind="ExternalInput")
    xT_d = nc.dram_tensor("xT", [128, 8, T], bf16, kind="ExternalInput")
    wkq_d = nc.dram_tensor("wkq", [128, 8, KQF], bf16, kind="ExternalInput")
    bkq_d = nc.dram_tensor("bkq", [128, 4], f32, kind="ExternalInput")
    wv_d = nc.dram_tensor("wv", [128, 8, CPC], bf16, kind="ExternalInput")
    wp_d = nc.dram_tensor("wp", [128, 2, C], bf16, kind="ExternalInput")
    mask_d = nc.dram_tensor("mask", [128, 2, 128], bf16, kind="ExternalInput")
    out_d = nc.dram_tensor("out", [T, C], bf16, kind="ExternalOutput")

    with tile.TileContext(nc) as tc, ExitStack() as ctx:
        pp = ctx.enter_context(tc.tile_pool(name="persist", bufs=1))
        xq0_sb = pp.tile([128, 8, 256 + TCH], bf16)
        xT_sb = pp.tile([128, 8, T], bf16)
        wkq_sb = pp.tile([128, 8, KQF], bf16)
        bkq_sb = pp.tile([128, 4], f32)
        wv_sb = pp.tile([128, 8, CPC], bf16)
        wp_sb = pp.tile([128, 2, C], bf16)
        mask_sb = pp.tile([128, 2, 128], bf16)
        kq_sb = pp.tile([128, 4, T], bf16)
        v_sb = pp.tile([128, NTI, HPC, D + 1], bf16)
        attn_sb = pp.tile([128, 2, T], bf16)

        # critical path first.  Host orders wkq features [K01, Q01, K23,
        # Q23], so the half the first head-pair needs is one strided DMA;
        # xT's first chunk goes per-c so KQ matmuls unlock progressively.
        # weights ride the Pool engine's SWDGE queue so their descriptor
        # generation runs in parallel with the xT stream's on HWDGE
        for c in range(8):
            nc.sync.dma_start(xq0_sb[:, c, :], xq0_d[:, c, :])
            if c == 0:
                nc.sync.dma_start(bkq_sb[:], bkq_d[:])
        nc.sync.dma_start(wv_sb[:], wv_d[:])
        nc.sync.dma_start(wkq_sb[:, :, 256:512], wkq_d[:, :, 256:512])
        for tch in range(1, NJ):
            nc.sync.dma_start(xT_sb[:, :, ts(tch, TCH)],
                              xT_d[:, :, ts(tch, TCH)])
        nc.sync.dma_start(wp_sb[:], wp_d[:])
        nc.sync.dma_start(mask_sb[:], mask_d[:])
        for ti in range(NTI):
            nc.any.memset(v_sb[:, ti, :, D:D + 1], 1.0)

        # PSUM: aff pairs 2x[128,2,512] (4 banks) + acc 2x[128,512]
        # (2 banks) + work 2x[128,512] (2 banks) = 8 banks.
        pa_pool = ctx.enter_context(
            tc.tile_pool(name="pall", bufs=1, space="PSUM"))
        e_pool = ctx.enter_context(tc.tile_pool(name="epool", bufs=1))
        r_pool = ctx.enter_context(tc.tile_pool(name="rpool", bufs=1))
        o_pool = ctx.enter_context(tc.tile_pool(name="outp", bufs=1))

        def wkq_at(c, m):
            # feature tiles m0/m1 arrive packed with xT chunk 0 in xq0
            if m < 2:
                return xq0_sb[:, c, ts(m, 128)]
            return wkq_sb[:, c, ts(m, 128)]

        def xT_at(c, t0, t1):
            if t1 <= TCH:
                return xq0_sb[:, c, 256 + t0:256 + t1]
            return xT_sb[:, c, t0:t1]

        def emit_kq_tiles(ms, tch):
            # interleave the c-loops of several feature tiles so the PE can
            # advance as each 128-row chunk of x arrives from HBM
            pk = {m: pa_pool.tile([128, TCH], f32, tag="work", bufs=2,
                                  name="pkq") for m in ms}
            for c in range(8):
                for m in ms:
                    nc.tensor.matmul(
                        pk[m][:], wkq_at(c, m),
                        xT_at(c, tch * TCH, (tch + 1) * TCH),
                        start=(c == 0), stop=(c == 7))
            for m in ms:
                nc.vector.tensor_scalar_add(
                    kq_sb[:, m, ts(tch, TCH)], pk[m][:], bkq_sb[:, m:m + 1])

        def emit_kq_tile(m, tch):
            emit_kq_tiles([m], tch)

        def emit_v(ti):
            pv = pa_pool.tile([128, CPC], f32, tag="work", bufs=2, name="pv")
            for c in range(8):
                nc.tensor.matmul(
                    pv[:], xT_at(c, ti * 128, (ti + 1) * 128), wv_sb[:, c, :],
                    start=(c == 0), stop=(c == 7))
            nc.vector.tensor_copy(
                v_sb[:, ti, :, 0:D],
                pv[:].rearrange("p (h d) -> p h d", h=HPC))

        def emit_attn_block(g, j, hooks=(), fillers=(), filler_start=2):
            """Emit one (head-pair, q-chunk) attention block.

            `hooks` run once at tile 1 (used for the previous block's
            deferred epilogue).  `fillers` are closures emitting ~1-2 PE
            matmuls each; they are drained one per tile from tile 4 on, so
            the PE has independent work wherever ScalarE exp would
            otherwise gate the AV matmuls.  Returns the epilogue closure.
            """
            pav0 = pa_pool.tile([128, TCH], f32, tag="acc", bufs=2,
                                name="pav0")
            pav1 = pa_pool.tile([128, TCH], f32, tag="acc", bufs=2,
                                name="pav1")
            n_live = 4 * j + 4
            es = {}
            fillq = list(fillers)

            def emit_aff(i):
                # diagonal tiles only touch queries q >= k: narrow the
                # q-range to [q0:TCH]
                q0 = max(0, 128 * i - TCH * j)
                qsl = slice(j * TCH + q0, (j + 1) * TCH)
                ap = pa_pool.tile([128, 2, TCH], f32, tag="aff", bufs=2,
                                  name="affp")
                nc.tensor.matmul(
                    ap[:, 0, q0:], kq_sb[0:64, 2 * g, ts(i, 128)],
                    kq_sb[0:64, 2 * g + 1, qsl], start=True, stop=True)
                nc.tensor.matmul(
                    ap[:, 1, q0:], kq_sb[64:128, 2 * g, ts(i, 128)],
                    kq_sb[64:128, 2 * g + 1, qsl], start=True, stop=True)
                ep = e_pool.tile([128, 2, TCH], bf16, tag="e", bufs=6,
                                 name="ep")
                nc.scalar.activation(ep[:, :, q0:], ap[:, :, q0:], Exp,
                                     scale=0.125)
                if q0 > 0 or i == 4 * j:
                    nc.gpsimd.tensor_mul(
                        ep[:, :, q0:q0 + 128], ep[:, :, q0:q0 + 128],
                        mask_sb[:])
                es[i] = (ep, q0)

            def emit_av(i):
                ep, q0 = es.pop(i)
                first, last = (i == 0), (i == n_live - 1)
                nc.tensor.matmul(
                    pav0[0:65, q0:], v_sb[:, i, 2 * g + 0, :],
                    ep[:, 0, q0:], start=first, stop=last)
                nc.tensor.matmul(
                    pav1[0:65, q0:], v_sb[:, i, 2 * g + 1, :],
                    ep[:, 1, q0:], start=first, stop=last)

            look = min(4, n_live)
            for i in range(n_live):
                emit_aff(i)
                if i == 1:
                    for h in hooks:
                        h()
                if i >= look:
                    emit_av(i - look)
                if fillq and i >= filler_start:
                    fillq.pop(0)()
            for i in range(n_live - look, n_live):
                emit_av(i)
                if fillq:
                    fillq.pop(0)()
            for f in fillq:
                f()

            def finalize_cols(sl):
                r2 = r_pool.tile([1, 2, TCH], f32, tag="r2", bufs=2)
                nc.vector.reciprocal(r2[0:1, 0, sl], pav0[64:65, sl])
                nc.vector.reciprocal(r2[0:1, 1, sl], pav1[64:65, sl])
                rb2 = r_pool.tile([128, 2, TCH], f32, tag="rb2", bufs=2)
                nc.gpsimd.partition_broadcast(rb2[:, :, sl], r2[0:1, :, sl],
                                              channels=128)
                qsl = slice(j * TCH + sl.start, j * TCH + sl.stop)
                nc.vector.tensor_mul(
                    attn_sb[0:64, g, qsl], pav0[0:64, sl], rb2[0:64, 0, sl])
                nc.vector.tensor_mul(
                    attn_sb[64:128, g, qsl], pav1[0:64, sl],
                    rb2[64:128, 1, sl])

            def finalize():
                finalize_cols(slice(0, TCH))

            finalize.cols = finalize_cols
            return finalize

        def proj_units(j, tail=False):
            # (ti, och) units; the two och halves share one [128,1024] SBUF
            # staging tile so each ti goes out as a single DMA.  At the
            # kernel tail the och0 copy runs on the (by then idle) ScalarE
            # so DVE keeps pace with the PE.
            units = []
            for ti in range(4 * j, 4 * j + 4):
                ots = {}

                def u(ti=ti, och=0, ots=ots):
                    if och == 0:
                        ots[0] = o_pool.tile([128, C], bf16, tag="ot",
                                             bufs=3, name="ot")
                    po = pa_pool.tile([128, 512], f32, tag="work", bufs=2,
                                      name="po")
                    nc.tensor.matmul(
                        po[:], attn_sb[:, 0, ts(ti, 128)],
                        wp_sb[:, 0, ts(och, 512)], start=True, stop=False)
                    nc.tensor.matmul(
                        po[:], attn_sb[:, 1, ts(ti, 128)],
                        wp_sb[:, 1, ts(och, 512)], start=False, stop=True)
                    if tail and och == 0:
                        nc.scalar.copy(ots[0][:, ts(och, 512)], po[:])
                    else:
                        nc.vector.tensor_copy(ots[0][:, ts(och, 512)], po[:])
                    if och == 1:
                        nc.sync.dma_start(out_d[ts(ti, 128), :], ots[0][:])
                for och in range(2):
                    units.append(
                        (lambda ti=ti, och=och, ots=ots:
                         u(ti=ti, och=och, ots=ots)))
            return units

        def kq_filler(m, tch):
            return lambda: emit_kq_tile(m, tch)

        def v_filler(ti):
            return lambda: emit_v(ti)

        # loop 1: KQ/V production interleaved with g0 attention blocks.
        # Feature-tile order is [K01, Q01, K23, Q23]: g0 needs m0/m1; g1's
        # m2/m3 chunks are deferred into loop 2 as just-in-time fillers
        # (only chunk 0 must exist before block (g1,0) starts).
        fin = None
        emit_kq_tiles([0, 1], 0)
        for tch in range(NJ):
            if tch == 0:
                for ti in range(4):
                    emit_v(ti)
            fillers = []
            if tch < NJ - 1:
                fillers.append(kq_filler(0, tch + 1))
                fillers.append(kq_filler(1, tch + 1))
                fillers.extend(v_filler(ti)
                               for ti in range(4 * tch + 4, 4 * tch + 8))
            else:
                fillers.append(kq_filler(2, 0))
                fillers.append(kq_filler(3, 0))
            hooks = [fin] if fin else []
            fin = emit_attn_block(0, tch, hooks=hooks, fillers=fillers,
                                  filler_start=2)

        # loop 2: g1 attention blocks ascending; each block's fillers are
        # the NEXT chunk's K23/Q23 tiles plus the projection units of the
        # previously finalized chunk.
        prev_j = None
        for j in range(NJ):
            hooks = [fin]
            fillers = []
            if j + 1 < NJ:
                fillers.append(kq_filler(2, j + 1))
                fillers.append(kq_filler(3, j + 1))
            if prev_j is not None:
                fillers.extend(proj_units(prev_j))
            # blocks whose first fillers are proj units (which wait on the
            # hoisted epilogue's DVE/Pool chain) start filling later
            fin = emit_attn_block(1, j, hooks=hooks, fillers=fillers,
                                  filler_start=2 if j + 1 < NJ else 4)
            prev_j = j
        # pipelined tail: normalize the last chunk 128 columns at a time,
        # launching each t-tile's projection as soon as its piece lands
        units = proj_units(NJ - 1, tail=True)
        for qb in range(4):
            fin.cols(slice(qb * 128, (qb + 1) * 128))
            units[2 * qb]()
            units[2 * qb + 1]()

    nc.compile()
    return nc


def _get_program():
    if "nc" not in _CACHE:
        _CACHE["nc"] = _build_program()
    return _CACHE["nc"]


def _host_mask():
    # mask[p, s, c] = 1.0 iff key-local p <= query-local c, duplicated over
    # s (the two heads of a pair share the mask)
    i = np.arange(128)[:, None]
    jj = np.arange(128)[None, :]
    m = (i <= jj).astype(ml_dtypes.bfloat16)
    return np.ascontiguousarray(
        np.broadcast_to(m[:, None, :], (128, 2, 128)))


def _shard_inputs(x, w_attn, b_attn, w_proj, b_proj):
    bf = ml_dtypes.bfloat16
    mask = _host_mask()
    in_maps = []
    for c in range(N_CORES):
        b, g = divmod(c, 4)
        hs = slice(g * CPC, (g + 1) * CPC)
        # xT: (C, T) -> (128, 8, T)
        xT = np.ascontiguousarray(
            x[b].T.reshape(8, 128, T).transpose(1, 0, 2)).astype(bf)
        # K block rows 0:C, Q rows C:2C, V rows 2C:3C of w_attn.  Feature
        # tiles ordered [K01, Q01, K23, Q23] so the first head pair's
        # K and Q are one contiguous half.
        Kr = w_attn[g * CPC:(g + 1) * CPC]
        Qr = w_attn[C + g * CPC:C + (g + 1) * CPC]
        wkq = np.concatenate([Kr[0:128], Qr[0:128],
                              Kr[128:256], Qr[128:256]], axis=0)
        # (KQF, C) -> transpose -> (C, KQF) -> (128, 8, KQF)
        wkq = np.ascontiguousarray(
            wkq.T.reshape(8, 128, KQF).transpose(1, 0, 2)).astype(bf)
        bK = b_attn[g * CPC:(g + 1) * CPC]
        bQ = b_attn[C + g * CPC:C + (g + 1) * CPC]
        bkq = np.concatenate([bK[0:128], bQ[0:128], bK[128:256],
                              bQ[128:256]])
        bkq = np.ascontiguousarray(bkq.reshape(4, 128).T).astype(np.float32)
        wv = w_attn[2 * C + g * CPC:2 * C + (g + 1) * CPC]  # (CPC, C)
        wv = np.ascontiguousarray(
            wv.T.reshape(8, 128, CPC).transpose(1, 0, 2)).astype(bf)
        wp = w_proj[:, hs].T  # (CPC, C)
        wp = np.ascontiguousarray(
            wp.reshape(2, 128, C).transpose(1, 0, 2)).astype(bf)
        xq0 = np.ascontiguousarray(
            np.concatenate([wkq[:, :, 0:256], xT[:, :, 0:TCH]], axis=2))
        in_maps.append({"xq0": xq0, "xT": xT, "wkq": wkq, "bkq": bkq,
                        "wv": wv, "wp": wp, "mask": mask})
    return in_maps


def kernel(x, w_attn, b_attn, w_proj, b_proj):
    from concourse.bass_utils import run_bass_kernel_spmd

    nc = _get_program()
    in_maps = _shard_inputs(x, w_attn, b_attn, w_proj, b_proj)
    res = run_bass_kernel_spmd(nc, in_maps, core_ids=list(range(N_CORES)))
    out = np.zeros((B, T, C), dtype=np.float32)
    for c in range(N_CORES):
        b = c // 4
        out[b] += res.results[c]["out"].astype(np.float32)
    # V-bias contribution folded out of the device kernel:
    # (attn + bv)^T @ wp  =  attn^T @ wp  +  (bv @ wp)
    bv_full = b_attn[2 * C:3 * C].astype(np.float64)
    bias_out = bv_full @ w_proj.T.astype(np.float64)
    out += (b_proj.astype(np.float64) + bias_out)[None, None, :].astype(
        np.float32)
    return out


# revision 50
# speedup vs baseline: 1.2956x; 1.0056x over previous
"""Causal multi-head self-attention block (B=2, T=2048, C=1024, H=16) on 8
Trainium2 NeuronCores.

Sharding: core c = 4*b + g handles batch b (2-way data parallel) and head
group g (4-way tensor parallel over the 16 heads -> 4 heads/core).
c_attn is column-sharded (each core computes K/Q/V features only for its 4
heads); c_proj is row-sharded (each core contracts its 4 heads' attn output
against the matching w_proj columns and emits a full-width partial output).
The 4 partial outputs per batch are summed on the host (+ b_proj).

Per-core device pipeline (all matmuls bf16 with fp32 PSUM accumulation):
  1. KQ^T = (w_kq x)        -> [feat, T] layout, feat on partitions
  2. V    = (x^T w_v^T)     -> [T, d] natural layout, augmented with a
     ones column so the AV matmul also yields the softmax denominators
  3. per head pair, per 512-wide q chunk, over live (causal) k tiles:
       aff^T[k,q] for both heads -> one [128, 2, 512] PSUM pair (2 banks)
       E = exp(0.125*aff^T)      -> ONE wide ScalarE activation per tile
                                    (strided view on diagonal tiles), bf16
       diagonal-tile mask multiply runs on the Pool engine
       [attn^T unnorm; sums] += V_aug.T E   (M=65, per head)
     epilogue: reciprocal rows (DVE), partition_broadcast (Pool),
     normalize (DVE).  No PE involvement in the epilogue.
  4. out_partial = attn^T.T w_proj_slice -> PSUM, DMA'd straight to HBM.

Because each engine executes its compiled instruction stream strictly in
order, emission order is scheduling: aff runs 2 tiles ahead of AV (hides
the ScalarE exp latency), block epilogues are deferred into the next
block, and filler matmuls (K/Q for the other head pair, V tiles, output
projection) are injected mid-block wherever ScalarE would otherwise be
the per-tile rate limiter.
"""

import os
import sys

for _p in ("/opt/trn_rl_repo",):
    if os.path.isdir(_p) and _p not in sys.path:
        sys.path.append(_p)

import numpy as np
import ml_dtypes

B, T, C, H, D = 2, 2048, 1024, 16, 64
N_CORES = 8
HPC = H // 4          # heads per core = 4
CPC = HPC * D         # attn feature cols per core = 256
KQF = 2 * CPC         # K+Q features per core = 512
TCH = 512             # q-chunk width
NJ = T // TCH         # 4 q chunks
NTI = T // 128        # 16 t tiles

_CACHE = {}


def _build_program():
    from contextlib import ExitStack

    import concourse.bass as bass
    import concourse.mybir as mybir
    import concourse.tile as tile
    from concourse import bacc
    from concourse.bass import ts

    f32 = mybir.dt.float32
    bf16 = mybir.dt.bfloat16
    Exp = mybir.ActivationFunctionType.Exp

    nc = bacc.Bacc("TRN2", target_bir_lowering=False, debug=False,
                   num_devices=N_CORES)

    xq0_d = nc.dram_tensor("xq0", [128, 8, 256 + TCH], bf16,
                           kind="ExternalInput")
    xT_d = nc.dram_tensor("xT", [128, 8, T], bf16, kind="ExternalInput")
    wkq_d = nc.dram_tensor("wkq", [128, 8, KQF], bf16, kind="ExternalInput")
    bkq_d = nc.dram_tensor("bkq", [128, 4], f32, kind="ExternalInput")
    wv_d = nc.dram_tensor("wv", [128, 8, CPC], bf16, kind="ExternalInput")
    wp_d = nc.dram_tensor("wp", [128, 2, C], bf16, kind="ExternalInput")
    mask_d = nc.dram_tensor("mask", [128, 2, 128], bf16, kind="ExternalInput")
    out_d = nc.dram_tensor("out", [T, C], bf16, kind="ExternalOutput")

    with tile.TileContext(nc) as tc, ExitStack() as ctx:
        pp = ctx.enter_context(tc.tile_pool(name="persist", bufs=1))
        xq0_sb = pp.tile([128, 8, 256 + TCH], bf16)
        xT_sb = pp.tile([128, 8, T], bf16)
        wkq_sb = pp.tile([128, 8, KQF], bf16)
        bkq_sb = pp.tile([128, 4], f32)
        wv_sb = pp.tile([128, 8, CPC], bf16)
        wp_sb = pp.tile([128, 2, C], bf16)
        mask_sb = pp.tile([128, 2, 128], bf16)
        kq_sb = pp.tile([128, 4, T], bf16)
        v_sb = pp.tile([128, NTI, HPC, D + 1], bf16)
        attn_sb = pp.tile([128, 2, T], bf16)

        # critical path first.  Host orders wkq features [K01, Q01, K23,
        # Q23], so the half the first head-pair needs is one strided DMA;
        # xT's first chunk goes per-c so KQ matmuls unlock progressively.
        # weights ride the Pool engine's SWDGE queue so their descriptor
        # generation runs in parallel with the xT stream's on HWDGE
        nc.sync.dma_start(xq0_sb[:, 0, :], xq0_d[:, 0, :])
        nc.sync.dma_start(bkq_sb[:], bkq_d[:])
        nc.sync.dma_start(xq0_sb[:, 1:4, :], xq0_d[:, 1:4, :])
        nc.sync.dma_start(xq0_sb[:, 4:8, :], xq0_d[:, 4:8, :])
        nc.sync.dma_start(wv_sb[:], wv_d[:])
        nc.sync.dma_start(wkq_sb[:, :, 256:512], wkq_d[:, :, 256:512])
        for tch in range(1, NJ):
            nc.sync.dma_start(xT_sb[:, :, ts(tch, TCH)],
                              xT_d[:, :, ts(tch, TCH)])
        nc.sync.dma_start(wp_sb[:], wp_d[:])
        nc.sync.dma_start(mask_sb[:], mask_d[:])
        for ti in range(NTI):
            nc.any.memset(v_sb[:, ti, :, D:D + 1], 1.0)

        # PSUM: aff pairs 2x[128,2,512] (4 banks) + acc 2x[128,512]
        # (2 banks) + work 2x[128,512] (2 banks) = 8 banks.
        pa_pool = ctx.enter_context(
            tc.tile_pool(name="pall", bufs=1, space="PSUM"))
        e_pool = ctx.enter_context(tc.tile_pool(name="epool", bufs=1))
        r_pool = ctx.enter_context(tc.tile_pool(name="rpool", bufs=1))
        o_pool = ctx.enter_context(tc.tile_pool(name="outp", bufs=1))

        def wkq_at(c, m):
            # feature tiles m0/m1 arrive packed with xT chunk 0 in xq0
            if m < 2:
                return xq0_sb[:, c, ts(m, 128)]
            return wkq_sb[:, c, ts(m, 128)]

        def xT_at(c, t0, t1):
            if t1 <= TCH:
                return xq0_sb[:, c, 256 + t0:256 + t1]
            return xT_sb[:, c, t0:t1]

        def emit_kq_tiles(ms, tch):
            # interleave the c-loops of several feature tiles so the PE can
            # advance as each 128-row chunk of x arrives from HBM
            pk = {m: pa_pool.tile([128, TCH], f32, tag="work", bufs=2,
                                  name="pkq") for m in ms}
            for c in range(8):
                for m in ms:
                    nc.tensor.matmul(
                        pk[m][:], wkq_at(c, m),
                        xT_at(c, tch * TCH, (tch + 1) * TCH),
                        start=(c == 0), stop=(c == 7))
            for m in ms:
                nc.vector.tensor_scalar_add(
                    kq_sb[:, m, ts(tch, TCH)], pk[m][:], bkq_sb[:, m:m + 1])

        def emit_kq_tile(m, tch):
            emit_kq_tiles([m], tch)

        def emit_v(ti):
            pv = pa_pool.tile([128, CPC], f32, tag="work", bufs=2, name="pv")
            for c in range(8):
                nc.tensor.matmul(
                    pv[:], xT_at(c, ti * 128, (ti + 1) * 128), wv_sb[:, c, :],
                    start=(c == 0), stop=(c == 7))
            nc.vector.tensor_copy(
                v_sb[:, ti, :, 0:D],
                pv[:].rearrange("p (h d) -> p h d", h=HPC))

        def emit_attn_block(g, j, hooks=(), fillers=(), filler_start=2,
                            tail_units=None):
            """Emit one (head-pair, q-chunk) attention block.

            `hooks` run once at tile 1 (used for the previous block's
            deferred epilogue).  `fillers` are closures emitting ~1-2 PE
            matmuls each; they are drained one per tile from tile 4 on, so
            the PE has independent work wherever ScalarE exp would
            otherwise gate the AV matmuls.  Returns the epilogue closure.
            """
            pav0 = pa_pool.tile([128, TCH], f32, tag="acc", bufs=2,
                                name="pav0")
            pav1 = pa_pool.tile([128, TCH], f32, tag="acc", bufs=2,
                                name="pav1")
            n_live = 4 * j + 4
            es = {}
            fillq = list(fillers)

            def emit_aff(i):
                # diagonal tiles only touch queries q >= k: narrow the
                # q-range to [q0:TCH]
                q0 = max(0, 128 * i - TCH * j)
                qsl = slice(j * TCH + q0, (j + 1) * TCH)
                ap = pa_pool.tile([128, 2, TCH], f32, tag="aff", bufs=2,
                                  name="affp")
                nc.tensor.matmul(
                    ap[:, 0, q0:], kq_sb[0:64, 2 * g, ts(i, 128)],
                    kq_sb[0:64, 2 * g + 1, qsl], start=True, stop=True)
                nc.tensor.matmul(
                    ap[:, 1, q0:], kq_sb[64:128, 2 * g, ts(i, 128)],
                    kq_sb[64:128, 2 * g + 1, qsl], start=True, stop=True)
                ep = e_pool.tile([128, 2, TCH], bf16, tag="e", bufs=6,
                                 name="ep")
                nc.scalar.activation(ep[:, :, q0:], ap[:, :, q0:], Exp,
                                     scale=0.125)
                if q0 > 0 or i == 4 * j:
                    nc.gpsimd.tensor_mul(
                        ep[:, :, q0:q0 + 128], ep[:, :, q0:q0 + 128],
                        mask_sb[:])
                es[i] = (ep, q0)

            def emit_av(i):
                ep, q0 = es.pop(i)
                first, last = (i == 0), (i == n_live - 1)
                nc.tensor.matmul(
                    pav0[0:65, q0:], v_sb[:, i, 2 * g + 0, :],
                    ep[:, 0, q0:], start=first, stop=last)
                nc.tensor.matmul(
                    pav1[0:65, q0:], v_sb[:, i, 2 * g + 1, :],
                    ep[:, 1, q0:], start=first, stop=last)

            def finalize_cols(sl):
                r2 = r_pool.tile([1, 2, TCH], f32, tag="r2", bufs=2)
                nc.vector.reciprocal(r2[0:1, 0, sl], pav0[64:65, sl])
                nc.vector.reciprocal(r2[0:1, 1, sl], pav1[64:65, sl])
                rb2 = r_pool.tile([128, 2, TCH], f32, tag="rb2", bufs=2)
                nc.gpsimd.partition_broadcast(rb2[:, :, sl], r2[0:1, :, sl],
                                              channels=128)
                qsl = slice(j * TCH + sl.start, j * TCH + sl.stop)
                nc.vector.tensor_mul(
                    attn_sb[0:64, g, qsl], pav0[0:64, sl], rb2[0:64, 0, sl])
                nc.vector.tensor_mul(
                    attn_sb[64:128, g, qsl], pav1[0:64, sl],
                    rb2[64:128, 1, sl])

            look = min(4, n_live)
            for i in range(n_live):
                emit_aff(i)
                if i == 1:
                    for h in hooks:
                        h()
                if i >= look:
                    emit_av(i - look)
                if fillq and i >= filler_start:
                    fillq.pop(0)()
            for i in range(n_live - look, n_live):
                emit_av(i)
                if tail_units is not None and i >= 4 * j:
                    # staircase completion: pav columns [qb*128:(qb+1)*128]
                    # are final right after av(4j+qb), so normalize each
                    # piece early; its projection follows one av later so
                    # the PE never waits on the piece's DVE/Pool chain
                    qb = i - 4 * j
                    finalize_cols(slice(qb * 128, (qb + 1) * 128))
                    if qb >= 1:
                        tail_units[2 * (qb - 1)]()
                        tail_units[2 * qb - 1]()
                if fillq:
                    fillq.pop(0)()
            for f in fillq:
                f()
            if tail_units is not None:
                tail_units[6]()
                tail_units[7]()

            def finalize():
                finalize_cols(slice(0, TCH))

            finalize.cols = finalize_cols
            return finalize

        def proj_units(j, tail=False):
            # (ti, och) units; the two och halves share one [128,1024] SBUF
            # staging tile so each ti goes out as a single DMA.  At the
            # kernel tail the och0 copy runs on the (by then idle) ScalarE
            # so DVE keeps pace with the PE.
            units = []
            for ti in range(4 * j, 4 * j + 4):
                ots = {}

                def u(ti=ti, och=0, ots=ots):
                    if och == 0:
                        ots[0] = o_pool.tile([128, C], bf16, tag="ot",
                                             bufs=3, name="ot")
                    po = pa_pool.tile([128, 512], f32, tag="work", bufs=2,
                                      name="po")
                    nc.tensor.matmul(
                        po[:], attn_sb[:, 0, ts(ti, 128)],
                        wp_sb[:, 0, ts(och, 512)], start=True, stop=False)
                    nc.tensor.matmul(
                        po[:], attn_sb[:, 1, ts(ti, 128)],
                        wp_sb[:, 1, ts(och, 512)], start=False, stop=True)
                    if tail and och == 0:
                        nc.scalar.copy(ots[0][:, ts(och, 512)], po[:])
                    else:
                        nc.vector.tensor_copy(ots[0][:, ts(och, 512)], po[:])
                    if och == 1:
                        nc.sync.dma_start(out_d[ts(ti, 128), :], ots[0][:])
                for och in range(2):
                    units.append(
                        (lambda ti=ti, och=och, ots=ots:
                         u(ti=ti, och=och, ots=ots)))
            return units

        def kq_filler(m, tch):
            return lambda: emit_kq_tile(m, tch)

        def v_filler(ti):
            return lambda: emit_v(ti)

        # loop 1: KQ/V production interleaved with g0 attention blocks.
        # Feature-tile order is [K01, Q01, K23, Q23]: g0 needs m0/m1; g1's
        # m2/m3 chunks are deferred into loop 2 as just-in-time fillers
        # (only chunk 0 must exist before block (g1,0) starts).
        fin = None
        emit_kq_tiles([0, 1], 0)
        for tch in range(NJ):
            if tch == 0:
                for ti in range(4):
                    emit_v(ti)
            fillers = []
            if tch < NJ - 1:
                fillers.append(kq_filler(0, tch + 1))
                fillers.append(kq_filler(1, tch + 1))
                fillers.extend(v_filler(ti)
                               for ti in range(4 * tch + 4, 4 * tch + 8))
            else:
                fillers.append(kq_filler(2, 0))
                fillers.append(kq_filler(3, 0))
            hooks = [fin] if fin else []
            fin = emit_attn_block(0, tch, hooks=hooks, fillers=fillers,
                                  filler_start=2)

        # loop 2: g1 attention blocks ascending; each block's fillers are
        # the NEXT chunk's K23/Q23 tiles plus the projection units of the
        # previously finalized chunk.
        prev_j = None
        for j in range(NJ):
            hooks = [fin]
            fillers = []
            if j + 1 < NJ:
                fillers.append(kq_filler(2, j + 1))
                fillers.append(kq_filler(3, j + 1))
            if prev_j is not None:
                fillers.extend(proj_units(prev_j))
            tail_units = (proj_units(NJ - 1, tail=True)
                          if j == NJ - 1 else None)
            # blocks whose first fillers are proj units (which wait on the
            # hoisted epilogue's DVE/Pool chain) start filling later
            fin = emit_attn_block(1, j, hooks=hooks, fillers=fillers,
                                  filler_start=2 if j + 1 < NJ else 4,
                                  tail_units=tail_units)
            prev_j = j

    nc.compile()
    return nc


def _get_program():
    if "nc" not in _CACHE:
        _CACHE["nc"] = _build_program()
    return _CACHE["nc"]


def _host_mask():
    # mask[p, s, c] = 1.0 iff key-local p <= query-local c, duplicated over
    # s (the two heads of a pair share the mask)
    i = np.arange(128)[:, None]
    jj = np.arange(128)[None, :]
    m = (i <= jj).astype(ml_dtypes.bfloat16)
    return np.ascontiguousarray(
        np.broadcast_to(m[:, None, :], (128, 2, 128)))


def _shard_inputs(x, w_attn, b_attn, w_proj, b_proj):
    bf = ml_dtypes.bfloat16
    mask = _host_mask()
    in_maps = []
    for c in range(N_CORES):
        b, g = divmod(c, 4)
        hs = slice(g * CPC, (g + 1) * CPC)
        # xT: (C, T) -> (128, 8, T)
        xT = np.ascontiguousarray(
            x[b].T.reshape(8, 128, T).transpose(1, 0, 2)).astype(bf)
        # K block rows 0:C, Q rows C:2C, V rows 2C:3C of w_attn.  Feature
        # tiles ordered [K01, Q01, K23, Q23] so the first head pair's
        # K and Q are one contiguous half.
        Kr = w_attn[g * CPC:(g + 1) * CPC]
        Qr = w_attn[C + g * CPC:C + (g + 1) * CPC]
        wkq = np.concatenate([Kr[0:128], Qr[0:128],
                              Kr[128:256], Qr[128:256]], axis=0)
        # (KQF, C) -> transpose -> (C, KQF) -> (128, 8, KQF)
        wkq = np.ascontiguousarray(
            wkq.T.reshape(8, 128, KQF).transpose(1, 0, 2)).astype(bf)
        bK = b_attn[g * CPC:(g + 1) * CPC]
        bQ = b_attn[C + g * CPC:C + (g + 1) * CPC]
        bkq = np.concatenate([bK[0:128], bQ[0:128], bK[128:256],
                              bQ[128:256]])
        bkq = np.ascontiguousarray(bkq.reshape(4, 128).T).astype(np.float32)
        wv = w_attn[2 * C + g * CPC:2 * C + (g + 1) * CPC]  # (CPC, C)
        wv = np.ascontiguousarray(
            wv.T.reshape(8, 128, CPC).transpose(1, 0, 2)).astype(bf)
        wp = w_proj[:, hs].T  # (CPC, C)
        wp = np.ascontiguousarray(
            wp.reshape(2, 128, C).transpose(1, 0, 2)).astype(bf)
        xq0 = np.ascontiguousarray(
            np.concatenate([wkq[:, :, 0:256], xT[:, :, 0:TCH]], axis=2))
        in_maps.append({"xq0": xq0, "xT": xT, "wkq": wkq, "bkq": bkq,
                        "wv": wv, "wp": wp, "mask": mask})
    return in_maps


def kernel(x, w_attn, b_attn, w_proj, b_proj):
    from concourse.bass_utils import run_bass_kernel_spmd

    nc = _get_program()
    in_maps = _shard_inputs(x, w_attn, b_attn, w_proj, b_proj)
    res = run_bass_kernel_spmd(nc, in_maps, core_ids=list(range(N_CORES)))
    out = np.zeros((B, T, C), dtype=np.float32)
    for c in range(N_CORES):
        b = c // 4
        out[b] += res.results[c]["out"].astype(np.float32)
    # V-bias contribution folded out of the device kernel:
    # (attn + bv)^T @ wp  =  attn^T @ wp  +  (bv @ wp)
    bv_full = b_attn[2 * C:3 * C].astype(np.float64)
    bias_out = bv_full @ w_proj.T.astype(np.float64)
    out += (b_proj.astype(np.float64) + bias_out)[None, None, :].astype(
        np.float32)
    return out


# revision 64
# speedup vs baseline: 1.3123x; 1.0129x over previous
"""Causal multi-head self-attention block (B=2, T=2048, C=1024, H=16) on 8
Trainium2 NeuronCores.

Sharding: core c = 4*b + g handles batch b (2-way data parallel) and head
group g (4-way tensor parallel over the 16 heads -> 4 heads/core).
c_attn is column-sharded (each core computes K/Q/V features only for its 4
heads); c_proj is row-sharded (each core contracts its 4 heads' attn output
against the matching w_proj columns and emits a full-width partial output).
The 4 partial outputs per batch are summed on the host (+ b_proj).

Per-core device pipeline (all matmuls bf16 with fp32 PSUM accumulation):
  1. KQ^T = (w_kq x)        -> [feat, T] layout, feat on partitions
  2. V    = (x^T w_v^T)     -> [T, d] natural layout, augmented with a
     ones column so the AV matmul also yields the softmax denominators
  3. per head pair, per 512-wide q chunk, over live (causal) k tiles:
       aff^T[k,q] for both heads -> one [128, 2, 512] PSUM pair (2 banks)
       E = exp(0.125*aff^T)      -> ONE wide ScalarE activation per tile
                                    (strided view on diagonal tiles), bf16
       diagonal-tile mask multiply runs on the Pool engine
       [attn^T unnorm; sums] += V_aug.T E   (M=65, per head)
     epilogue: reciprocal rows (DVE), partition_broadcast (Pool),
     normalize (DVE).  No PE involvement in the epilogue.
  4. out_partial = attn^T.T w_proj_slice -> PSUM, DMA'd straight to HBM.

Because each engine executes its compiled instruction stream strictly in
order, emission order is scheduling: aff runs 2 tiles ahead of AV (hides
the ScalarE exp latency), block epilogues are deferred into the next
block, and filler matmuls (K/Q for the other head pair, V tiles, output
projection) are injected mid-block wherever ScalarE would otherwise be
the per-tile rate limiter.
"""

import os
import sys

for _p in ("/opt/trn_rl_repo",):
    if os.path.isdir(_p) and _p not in sys.path:
        sys.path.append(_p)

import numpy as np
import ml_dtypes

B, T, C, H, D = 2, 2048, 1024, 16, 64
N_CORES = 8
HPC = H // 4          # heads per core = 4
CPC = HPC * D         # attn feature cols per core = 256
KQF = 2 * CPC         # K+Q features per core = 512
TCH = 512             # q-chunk width
NJ = T // TCH         # 4 q chunks
NTI = T // 128        # 16 t tiles

_CACHE = {}


def _build_program():
    from contextlib import ExitStack

    import concourse.bass as bass
    import concourse.mybir as mybir
    import concourse.tile as tile
    from concourse import bacc
    from concourse.bass import ts

    f32 = mybir.dt.float32
    bf16 = mybir.dt.bfloat16
    Exp = mybir.ActivationFunctionType.Exp

    nc = bacc.Bacc("TRN2", target_bir_lowering=False, debug=False,
                   num_devices=N_CORES)

    xq0_d = nc.dram_tensor("xq0", [128, 8, 256 + TCH], bf16,
                           kind="ExternalInput")
    xT_d = nc.dram_tensor("xT", [128, 8, T], bf16, kind="ExternalInput")
    wkq_d = nc.dram_tensor("wkq", [128, 8, KQF], bf16, kind="ExternalInput")
    bkq_d = nc.dram_tensor("bkq", [128, 4], f32, kind="ExternalInput")
    wv_d = nc.dram_tensor("wv", [128, 8, CPC], bf16, kind="ExternalInput")
    wp_d = nc.dram_tensor("wp", [128, 2, C], bf16, kind="ExternalInput")
    mask_d = nc.dram_tensor("mask", [128, 2, 128], bf16, kind="ExternalInput")
    out_d = nc.dram_tensor("out", [T, C], bf16, kind="ExternalOutput")

    with tile.TileContext(nc) as tc, ExitStack() as ctx:
        pp = ctx.enter_context(tc.tile_pool(name="persist", bufs=1))
        xq0_sb = pp.tile([128, 8, 256 + TCH], bf16)
        xT_sb = pp.tile([128, 8, T], bf16)
        wkq_sb = pp.tile([128, 8, KQF], bf16)
        bkq_sb = pp.tile([128, 4], f32)
        wv_sb = pp.tile([128, 8, CPC], bf16)
        wp_sb = pp.tile([128, 2, C], bf16)
        mask_sb = pp.tile([128, 2, 128], bf16)
        kq_sb = pp.tile([128, 4, T], bf16)
        v_sb = pp.tile([128, NTI, HPC, D + 1], bf16)
        attn_sb = pp.tile([128, 2, T], bf16)

        # critical path first.  Host orders wkq features [K01, Q01, K23,
        # Q23], so the half the first head-pair needs is one strided DMA;
        # xT's first chunk goes per-c so KQ matmuls unlock progressively.
        # weights ride the Pool engine's SWDGE queue so their descriptor
        # generation runs in parallel with the xT stream's on HWDGE
        nc.sync.dma_start(xq0_sb[:, 0, :], xq0_d[:, 0, :])
        nc.sync.dma_start(bkq_sb[:], bkq_d[:])
        nc.sync.dma_start(xq0_sb[:, 1:4, :], xq0_d[:, 1:4, :])
        nc.sync.dma_start(xq0_sb[:, 4:8, :], xq0_d[:, 4:8, :])
        nc.sync.dma_start(wv_sb[:], wv_d[:])
        nc.sync.dma_start(wkq_sb[:, :, 256:512], wkq_d[:, :, 256:512])
        for tch in range(1, NJ):
            nc.sync.dma_start(xT_sb[:, :, ts(tch, TCH)],
                              xT_d[:, :, ts(tch, TCH)])
        nc.sync.dma_start(wp_sb[:], wp_d[:])
        nc.sync.dma_start(mask_sb[:], mask_d[:])
        for ti in range(NTI):
            nc.any.memset(v_sb[:, ti, :, D:D + 1], 1.0)

        # PSUM: aff pairs 2x[128,2,512] (4 banks) + acc 2x[128,512]
        # (2 banks) + work 2x[128,512] (2 banks) = 8 banks.
        pa_pool = ctx.enter_context(
            tc.tile_pool(name="pall", bufs=1, space="PSUM"))
        e_pool = ctx.enter_context(tc.tile_pool(name="epool", bufs=1))
        r_pool = ctx.enter_context(tc.tile_pool(name="rpool", bufs=1))
        o_pool = ctx.enter_context(tc.tile_pool(name="outp", bufs=1))

        def wkq_at(c, m):
            # feature tiles m0/m1 arrive packed with xT chunk 0 in xq0
            if m < 2:
                return xq0_sb[:, c, ts(m, 128)]
            return wkq_sb[:, c, ts(m, 128)]

        def xT_at(c, t0, t1):
            if t1 <= TCH:
                return xq0_sb[:, c, 256 + t0:256 + t1]
            return xT_sb[:, c, t0:t1]

        def emit_kq_tiles(ms, tch):
            # interleave the c-loops of several feature tiles so the PE can
            # advance as each 128-row chunk of x arrives from HBM
            pk = {m: pa_pool.tile([128, TCH], f32, tag="work", bufs=2,
                                  name="pkq") for m in ms}
            for c in range(8):
                for m in ms:
                    nc.tensor.matmul(
                        pk[m][:], wkq_at(c, m),
                        xT_at(c, tch * TCH, (tch + 1) * TCH),
                        start=(c == 0), stop=(c == 7))
            for m in ms:
                nc.vector.tensor_scalar_add(
                    kq_sb[:, m, ts(tch, TCH)], pk[m][:], bkq_sb[:, m:m + 1])

        def emit_kq_tile(m, tch):
            emit_kq_tiles([m], tch)

        def emit_v(ti):
            pv = pa_pool.tile([128, CPC], f32, tag="work", bufs=2, name="pv")
            for c in range(8):
                nc.tensor.matmul(
                    pv[:], xT_at(c, ti * 128, (ti + 1) * 128), wv_sb[:, c, :],
                    start=(c == 0), stop=(c == 7))
            nc.vector.tensor_copy(
                v_sb[:, ti, :, 0:D],
                pv[:].rearrange("p (h d) -> p h d", h=HPC))

        def emit_attn_block(g, j, hooks=(), fillers=(), filler_start=2,
                            tail_units=None):
            """Emit one (head-pair, q-chunk) attention block.

            `hooks` run once at tile 1 (used for the previous block's
            deferred epilogue).  `fillers` are closures emitting ~1-2 PE
            matmuls each; they are drained one per tile from tile 4 on, so
            the PE has independent work wherever ScalarE exp would
            otherwise gate the AV matmuls.  Returns the epilogue closure.
            """
            pav0 = pa_pool.tile([128, TCH], f32, tag="acc", bufs=2,
                                name="pav0")
            pav1 = pa_pool.tile([128, TCH], f32, tag="acc", bufs=2,
                                name="pav1")
            n_live = 4 * j + 4
            es = {}
            fillq = list(fillers)

            def emit_aff(i):
                # diagonal tiles only touch queries q >= k: narrow the
                # q-range to [q0:TCH]
                q0 = max(0, 128 * i - TCH * j)
                qsl = slice(j * TCH + q0, (j + 1) * TCH)
                ap = pa_pool.tile([128, 2, TCH], f32, tag="aff", bufs=2,
                                  name="affp")
                nc.tensor.matmul(
                    ap[:, 0, q0:], kq_sb[0:64, 2 * g, ts(i, 128)],
                    kq_sb[0:64, 2 * g + 1, qsl], start=True, stop=True)
                nc.tensor.matmul(
                    ap[:, 1, q0:], kq_sb[64:128, 2 * g, ts(i, 128)],
                    kq_sb[64:128, 2 * g + 1, qsl], start=True, stop=True)
                ep = e_pool.tile([128, 2, TCH], bf16, tag="e", bufs=6,
                                 name="ep")
                nc.scalar.activation(ep[:, :, q0:], ap[:, :, q0:], Exp,
                                     scale=0.125)
                if q0 > 0 or i == 4 * j:
                    nc.gpsimd.tensor_mul(
                        ep[:, :, q0:q0 + 128], ep[:, :, q0:q0 + 128],
                        mask_sb[:])
                es[i] = (ep, q0)

            def emit_av(i):
                ep, q0 = es.pop(i)
                first, last = (i == 0), (i == n_live - 1)
                nc.tensor.matmul(
                    pav0[0:65, q0:], v_sb[:, i, 2 * g + 0, :],
                    ep[:, 0, q0:], start=first, stop=last)
                nc.tensor.matmul(
                    pav1[0:65, q0:], v_sb[:, i, 2 * g + 1, :],
                    ep[:, 1, q0:], start=first, stop=last)

            def finalize_cols(sl):
                r2 = r_pool.tile([1, 2, TCH], f32, tag="r2", bufs=2)
                nc.vector.reciprocal(r2[0:1, 0, sl], pav0[64:65, sl])
                nc.vector.reciprocal(r2[0:1, 1, sl], pav1[64:65, sl])
                rb2 = r_pool.tile([128, 2, TCH], f32, tag="rb2", bufs=2)
                nc.gpsimd.partition_broadcast(rb2[:, :, sl], r2[0:1, :, sl],
                                              channels=128)
                qsl = slice(j * TCH + sl.start, j * TCH + sl.stop)
                nc.vector.tensor_mul(
                    attn_sb[0:64, g, qsl], pav0[0:64, sl], rb2[0:64, 0, sl])
                nc.vector.tensor_mul(
                    attn_sb[64:128, g, qsl], pav1[0:64, sl],
                    rb2[64:128, 1, sl])

            look = min(4, n_live)
            for i in range(n_live):
                emit_aff(i)
                if i == 1:
                    for h in hooks:
                        h()
                if i >= look:
                    emit_av(i - look)
                if fillq and i >= filler_start:
                    fillq.pop(0)()
            for i in range(n_live - look, n_live):
                emit_av(i)
                if tail_units is not None and i >= 4 * j:
                    # staircase completion: pav columns [qb*128:(qb+1)*128]
                    # are final right after av(4j+qb), so normalize each
                    # piece early; its projection follows one av later so
                    # the PE never waits on the piece's DVE/Pool chain
                    qb = i - 4 * j
                    finalize_cols(slice(qb * 128, (qb + 1) * 128))
                    if qb >= 2:
                        tail_units[2 * (qb - 2)]()
                        tail_units[2 * qb - 3]()
                if fillq:
                    fillq.pop(0)()
            for f in fillq:
                f()
            if tail_units is not None:
                for u in tail_units[4:8]:
                    u()

            def finalize():
                finalize_cols(slice(0, TCH))

            finalize.cols = finalize_cols
            return finalize

        def proj_units(j, tail=False):
            # (ti, och) units; the two och halves share one [128,1024] SBUF
            # staging tile so each ti goes out as a single DMA.  At the
            # kernel tail the och0 copy runs on the (by then idle) ScalarE
            # so DVE keeps pace with the PE.
            units = []
            for ti in range(4 * j, 4 * j + 4):
                ots = {}

                def u(ti=ti, och=0, ots=ots):
                    if och == 0:
                        ots[0] = o_pool.tile([128, C], bf16, tag="ot",
                                             bufs=3, name="ot")
                    po = pa_pool.tile([128, 512], f32, tag="work", bufs=2,
                                      name="po")
                    nc.tensor.matmul(
                        po[:], attn_sb[:, 0, ts(ti, 128)],
                        wp_sb[:, 0, ts(och, 512)], start=True, stop=False)
                    nc.tensor.matmul(
                        po[:], attn_sb[:, 1, ts(ti, 128)],
                        wp_sb[:, 1, ts(och, 512)], start=False, stop=True)
                    if tail and och == 0:
                        nc.scalar.copy(ots[0][:, ts(och, 512)], po[:])
                    else:
                        nc.vector.tensor_copy(ots[0][:, ts(och, 512)], po[:])
                    if tail:
                        nc.sync.dma_start(
                            out_d[ts(ti, 128), ts(och, 512)],
                            ots[0][:, ts(och, 512)])
                    elif och == 1:
                        nc.sync.dma_start(out_d[ts(ti, 128), :], ots[0][:])
                for och in range(2):
                    units.append(
                        (lambda ti=ti, och=och, ots=ots:
                         u(ti=ti, och=och, ots=ots)))
            return units

        def kq_filler(m, tch):
            return lambda: emit_kq_tile(m, tch)

        def v_filler(ti):
            return lambda: emit_v(ti)

        # loop 1: KQ/V production interleaved with g0 attention blocks.
        # Feature-tile order is [K01, Q01, K23, Q23]: g0 needs m0/m1; g1's
        # m2/m3 chunks are deferred into loop 2 as just-in-time fillers
        # (only chunk 0 must exist before block (g1,0) starts).
        fin = None
        emit_kq_tiles([0, 1], 0)
        for tch in range(NJ):
            if tch == 0:
                for ti in range(4):
                    emit_v(ti)
            fillers = []
            if tch < NJ - 1:
                fillers.append(kq_filler(0, tch + 1))
                fillers.append(kq_filler(1, tch + 1))
                fillers.extend(v_filler(ti)
                               for ti in range(4 * tch + 4, 4 * tch + 8))
            else:
                fillers.append(kq_filler(2, 0))
                fillers.append(kq_filler(3, 0))
            hooks = [fin] if fin else []
            fin = emit_attn_block(0, tch, hooks=hooks, fillers=fillers,
                                  filler_start=2)

        # loop 2: g1 attention blocks ascending; each block's fillers are
        # the NEXT chunk's K23/Q23 tiles plus the projection units of the
        # previously finalized chunk.
        prev_j = None
        for j in range(NJ):
            hooks = [fin]
            fillers = []
            if j + 1 < NJ:
                fillers.append(kq_filler(2, j + 1))
                fillers.append(kq_filler(3, j + 1))
            if prev_j is not None:
                fillers.extend(proj_units(prev_j))
            tail_units = (proj_units(NJ - 1, tail=True)
                          if j == NJ - 1 else None)
            # blocks whose first fillers are proj units (which wait on the
            # hoisted epilogue's DVE/Pool chain) start filling later
            fin = emit_attn_block(1, j, hooks=hooks, fillers=fillers,
                                  filler_start=2 if j + 1 < NJ else 8,
                                  tail_units=tail_units)
            prev_j = j

    nc.compile()
    return nc


def _get_program():
    if "nc" not in _CACHE:
        _CACHE["nc"] = _build_program()
    return _CACHE["nc"]


def _host_mask():
    # mask[p, s, c] = 1.0 iff key-local p <= query-local c, duplicated over
    # s (the two heads of a pair share the mask)
    i = np.arange(128)[:, None]
    jj = np.arange(128)[None, :]
    m = (i <= jj).astype(ml_dtypes.bfloat16)
    return np.ascontiguousarray(
        np.broadcast_to(m[:, None, :], (128, 2, 128)))


def _shard_inputs(x, w_attn, b_attn, w_proj, b_proj):
    bf = ml_dtypes.bfloat16
    mask = _host_mask()
    in_maps = []
    for c in range(N_CORES):
        b, g = divmod(c, 4)
        hs = slice(g * CPC, (g + 1) * CPC)
        # xT: (C, T) -> (128, 8, T)
        xT = np.ascontiguousarray(
            x[b].T.reshape(8, 128, T).transpose(1, 0, 2)).astype(bf)
        # K block rows 0:C, Q rows C:2C, V rows 2C:3C of w_attn.  Feature
        # tiles ordered [K01, Q01, K23, Q23] so the first head pair's
        # K and Q are one contiguous half.
        Kr = w_attn[g * CPC:(g + 1) * CPC]
        Qr = w_attn[C + g * CPC:C + (g + 1) * CPC]
        wkq = np.concatenate([Kr[0:128], Qr[0:128],
                              Kr[128:256], Qr[128:256]], axis=0)
        # (KQF, C) -> transpose -> (C, KQF) -> (128, 8, KQF)
        wkq = np.ascontiguousarray(
            wkq.T.reshape(8, 128, KQF).transpose(1, 0, 2)).astype(bf)
        bK = b_attn[g * CPC:(g + 1) * CPC]
        bQ = b_attn[C + g * CPC:C + (g + 1) * CPC]
        bkq = np.concatenate([bK[0:128], bQ[0:128], bK[128:256],
                              bQ[128:256]])
        bkq = np.ascontiguousarray(bkq.reshape(4, 128).T).astype(np.float32)
        wv = w_attn[2 * C + g * CPC:2 * C + (g + 1) * CPC]  # (CPC, C)
        wv = np.ascontiguousarray(
            wv.T.reshape(8, 128, CPC).transpose(1, 0, 2)).astype(bf)
        wp = w_proj[:, hs].T  # (CPC, C)
        wp = np.ascontiguousarray(
            wp.reshape(2, 128, C).transpose(1, 0, 2)).astype(bf)
        xq0 = np.ascontiguousarray(
            np.concatenate([wkq[:, :, 0:256], xT[:, :, 0:TCH]], axis=2))
        in_maps.append({"xq0": xq0, "xT": xT, "wkq": wkq, "bkq": bkq,
                        "wv": wv, "wp": wp, "mask": mask})
    return in_maps


def kernel(x, w_attn, b_attn, w_proj, b_proj):
    from concourse.bass_utils import run_bass_kernel_spmd

    nc = _get_program()
    in_maps = _shard_inputs(x, w_attn, b_attn, w_proj, b_proj)
    res = run_bass_kernel_spmd(nc, in_maps, core_ids=list(range(N_CORES)))
    out = np.zeros((B, T, C), dtype=np.float32)
    for c in range(N_CORES):
        b = c // 4
        out[b] += res.results[c]["out"].astype(np.float32)
    # V-bias contribution folded out of the device kernel:
    # (attn + bv)^T @ wp  =  attn^T @ wp  +  (bv @ wp)
    bv_full = b_attn[2 * C:3 * C].astype(np.float64)
    bias_out = bv_full @ w_proj.T.astype(np.float64)
    out += (b_proj.astype(np.float64) + bias_out)[None, None, :].astype(
        np.float32)
    return out
